# revision 1
# baseline (speedup 1.0000x reference)
"""GNN (3x TransformerConv + BN + pooling + MLP) with layer-1 node
projections computed on 8 Trainium2 cores (row-sharded dense matmuls),
remaining graph ops on host. Self-contained: shapes hardcoded."""
import math
import numpy as np
from concourse import bacc, bass, tile, mybir
from concourse.bass_utils import run_bass_kernel_spmd

P = 8
N, E, F_IN, ED, G = 20000, 640000, 128, 4, 64
HC = 256
NLOC = N // P            # 2500 rows per core
NPAD = 2560              # 20 chunks of 128
NCH = NPAD // 128
EPS = 1e-5
F32 = mybir.dt.float32

LAST_EXEC_NS = None


def _build_program():
    nc = bacc.Bacc("TRN2", debug=False, num_devices=P)
    xm = nc.dram_tensor("xm", [NPAD, F_IN], F32, kind="ExternalInput")
    w4 = nc.dram_tensor("w4", [F_IN, 4 * HC], F32, kind="ExternalInput")
    b4 = nc.dram_tensor("b4", [1, 4 * HC], F32, kind="ExternalInput")
    idn = nc.dram_tensor("idn", [128, 128], F32, kind="ExternalInput")
    proj = nc.dram_tensor("proj", [NPAD, 4 * HC], F32, kind="ExternalOutput")
    with tile.TileContext(nc) as tc:
        with (
            tc.tile_pool(name="sb", bufs=1) as sb,
            tc.tile_pool(name="sb2", bufs=2) as sb2,
            tc.tile_pool(name="ps", bufs=2, space="PSUM") as ps,
        ):
            s_w = sb.tile([128, 4 * HC], F32, name="s_w", tag="s_w")
            nc.sync.dma_start(s_w[:], w4[:])
            s_b = sb.tile([128, 4 * HC], F32, name="s_b", tag="s_b")
            b_ap = b4[:]
            bb = bass.AP(tensor=b_ap.tensor, offset=b_ap.offset,
                         ap=[[0, 128], b_ap.ap[1]])
            nc.gpsimd.dma_start(s_b[:], bb)
            s_i = sb.tile([128, 128], F32, name="s_i", tag="s_i")
            nc.sync.dma_start(s_i[:], idn[:])
            xm_f = xm[:]
            pr_f = proj[:]
            for c in range(NCH):
                xc = sb2.tile([128, F_IN], F32, name="xc", tag="xc")
                nc.sync.dma_start(xc[:], xm_f[c * 128:(c + 1) * 128, :])
                pt = ps.tile([128, 128], F32, name="pt", tag="pt")
                nc.tensor.transpose(pt[:], xc[:], s_i[:])
                xT = sb2.tile([128, 128], F32, name="xT", tag="xT")
                nc.scalar.copy(xT[:], pt[:])
                ot = sb2.tile([128, 4 * HC], F32, name="ot", tag="ot")
                for h in range(2):
                    pm = ps.tile([128, 512], F32, name=f"pm{h}", tag=f"pm{h}")
                    nc.tensor.matmul(pm[:], xT[:], s_w[:, h * 512:(h + 1) * 512],
                                     start=True, stop=True)
                    nc.scalar.copy(ot[:, h * 512:(h + 1) * 512], pm[:])
                nc.vector.tensor_tensor(ot[:], ot[:], s_b[:], mybir.AluOpType.add)
                nc.sync.dma_start(pr_f[c * 128:(c + 1) * 128, :], ot[:])
    nc.finalize()
    return nc


def _device_proj1(x, q1w, q1b, k1w, k1b, v1w, v1b, s1w, s1b):
    global LAST_EXEC_NS
    nc = _build_program()
    w4 = np.concatenate([q1w, k1w, v1w, s1w], axis=1).astype(np.float32)
    b4 = np.concatenate([q1b, k1b, v1b, s1b])[None, :].astype(np.float32)
    idn = np.eye(128, dtype=np.float32)
    in_maps = []
    for m in range(P):
        xm = np.zeros((NPAD, F_IN), np.float32)
        xm[:NLOC] = x[m * NLOC:(m + 1) * NLOC]
        in_maps.append({"xm": xm, "w4": w4, "b4": b4, "idn": idn})
    import os
    import time
    res = run_bass_kernel_spmd(nc, in_maps, list(range(P)))
    LAST_EXEC_NS = res.exec_time_ns
    if LAST_EXEC_NS is None and os.environ.get("BASS_GNN_TIME") == "1":
        # NTFF profiling unavailable under this axon build; warm-cache
        # wall-clock of a second dispatch is the closest available proxy.
        t0 = time.perf_counter_ns()
        run_bass_kernel_spmd(nc, in_maps, list(range(P)))
        LAST_EXEC_NS = time.perf_counter_ns() - t0
    full = np.concatenate(
        [np.asarray(res.results[m]["proj"]).reshape(NPAD, 4 * HC)[:NLOC]
         for m in range(P)], axis=0)
    return (full[:, 0:HC], full[:, HC:2 * HC],
            full[:, 2 * HC:3 * HC], full[:, 3 * HC:4 * HC])


def _seg_sum_sorted(vals, starts, counts):
    st = np.minimum(starts, max(len(vals) - 1, 0))
    out = np.add.reduceat(vals, st, axis=0)
    out[counts == 0] = 0
    return out


def _seg_max_sorted(vals, starts, counts):
    st = np.minimum(starts, max(len(vals) - 1, 0))
    out = np.maximum.reduceat(vals, st, axis=0)
    out[counts == 0] = 0
    return out


def _tconv(x, src, dst, ea_e, H, C, qkvs=None, x_w=None, order=None,
           starts=None, counts=None):
    n = x.shape[0]
    if qkvs is not None:
        q, k, v, s = qkvs
    else:
        qw, qb, kw, kb, vw, vb, sw, sb_ = x_w
        q = x @ qw + qb
        k = x @ kw + kb
        v = x @ vw + vb
        s = x @ sw + sb_
    q = q.reshape(n, H, C)
    k = k.reshape(n, H, C)
    v = v.reshape(n, H, C)
    eh = ea_e.reshape(-1, H, C)[order]
    so, do = src[order], dst[order]
    kj = k[so] + eh
    alpha = np.einsum('ehc,ehc->eh', q[do], kj, dtype=np.float32) / math.sqrt(C)
    del kj
    amax = _seg_max_sorted(alpha, starts, counts)
    al = np.exp(alpha - amax[do])
    denom = _seg_sum_sorted(al, starts, counts)
    al = al / (denom[do] + 1e-16)
    msg = (v[so] + eh) * al[:, :, None]
    out = _seg_sum_sorted(msg.reshape(-1, H * C), starts, counts)
    del msg
    return out + s


def _bn(x, w, b):
    mu = x.mean(axis=0, dtype=np.float64).astype(np.float32)
    var = ((x - mu) ** 2).mean(axis=0, dtype=np.float64).astype(np.float32)
    return (x - mu) / np.sqrt(var + EPS) * w + b


def kernel(x, edge_index, edge_attr, batch,
           q1w, q1b, k1w, k1b, v1w, v1b, e1w, s1w, s1b, bn1w, bn1b,
           q2w, q2b, k2w, k2b, v2w, v2b, e2w, s2w, s2b, bn2w, bn2b,
           q3w, q3b, k3w, k3b, v3w, v3b, e3w, s3w, s3b, bn3w, bn3b,
           m1w, m1b, pa, m2w, m2b):
    x = np.asarray(x, np.float32)
    edge_index = np.asarray(edge_index)
    edge_attr = np.asarray(edge_attr, np.float32)
    batch = np.asarray(batch)
    src, dst = edge_index[0], edge_index[1]

    order = np.argsort(dst, kind="stable")
    counts = np.bincount(dst, minlength=N)
    starts = np.zeros(N, np.int64)
    starts[1:] = np.cumsum(counts)[:-1]

    Q1, K1, V1, S1 = _device_proj1(x, q1w, q1b, k1w, k1b, v1w, v1b, s1w, s1b)

    x1 = _bn(_tconv(x, src, dst, edge_attr @ e1w, 4, 64,
                    qkvs=(Q1, K1, V1, S1), order=order, starts=starts,
                    counts=counts), bn1w, bn1b)
    x2 = _bn(_tconv(x1, src, dst, edge_attr @ e2w, 1, HC,
                    x_w=(q2w, q2b, k2w, k2b, v2w, v2b, s2w, s2b),
                    order=order, starts=starts, counts=counts), bn2w, bn2b)
    x3 = _bn(_tconv(x2, src, dst, edge_attr @ e3w, 1, HC,
                    x_w=(q3w, q3b, k3w, k3b, v3w, v3b, s3w, s3b),
                    order=order, starts=starts, counts=counts), bn3w, bn3b)

    gcnt = np.bincount(batch, minlength=G)
    gstarts = np.zeros(G, np.int64)
    gstarts[1:] = np.cumsum(gcnt)[:-1]
    x_add = _seg_sum_sorted(x3, gstarts, gcnt)
    x_max = _seg_max_sorted(x3, gstarts, gcnt)
    x_mean = x_add / np.maximum(gcnt, 1)[:, None]
    h = np.concatenate([x_add, x_max, x_mean], axis=1).astype(np.float32)
    h = h @ m1w + m1b
    h = np.where(h >= 0, h, np.float32(pa) * h)
    lg = h @ m2w + m2b
    mx = lg.max(axis=1, keepdims=True)
    sh = lg - mx
    return (sh - np.log(np.exp(sh).sum(axis=1, keepdims=True))).astype(np.float32)



# revision 17
# speedup vs baseline: 5.5110x; 5.5110x over previous
"""Full 3-layer TransformerConv GNN on 8 Trainium2 cores.

Sharding: edges sorted by dst and partitioned into 8 contiguous dst-node
ranges (2500 nodes/core, padded to 2560).  Node projections (q/k/v/skip)
are computed replicated on every core into global DRAM tables; each core
runs segment-softmax message aggregation only for its 20 local 128-node
dst windows via one-hot scatter matmuls (PSUM-accumulated per window).
Pre-BN layer outputs are AllGathered (feat-major) between layers; BN
statistics are computed replicated from the gathered tensor.  Per-graph
sum/max/mean pooling happens on device; only the [64,768] pooled tensor
returns to the host, which applies the tiny MLP head.

All host->device payload travels in ONE packed fp16 tensor per core
(int16 index sections bitcast) to minimize axon-tunnel transfer time,
which dominates the dispatch wall clock.  Device compute stays fp32.

Self-contained: shapes hardcoded, sharding derived from the inputs.
"""
import math
import os
import time
import numpy as np

import jax
from jax.sharding import Mesh, PartitionSpec
from jax.experimental.shard_map import shard_map

from concourse import bacc, bass, tile, mybir, library_config

P = 8
N, E, F_IN, ED, G = 20000, 640000, 128, 4, 64
HC = 256
NLOC = N // P          # 2500
NWIN = 20              # 128-node dst windows per core
NPL = NWIN * 128       # 2560 padded local nodes
NT = P * NPL           # 20480 padded global nodes
EPS = 1e-5
F32 = mybir.dt.float32
F16 = mybir.dt.float16
I16 = mybir.dt.int16

LAST_EXEC_NS = None

_CACHE = {}


# ----------------------------------------------------------------- host pack
def _pack_edges(src, dst, edge_attr):
    order = np.argsort(dst, kind="stable")
    so = src[order].astype(np.int64)
    do = dst[order].astype(np.int64)
    eao = edge_attr[order].astype(np.float32)

    core = do // NLOC
    wloc = (do - core * NLOC) >> 7          # local window [0, NWIN)
    cw = np.zeros((P, NWIN), np.int64)
    np.add.at(cw, (core, wloc), 1)
    nblk = (-(-cw // 128)).max(axis=0)      # common blocks per window
    wstart = np.zeros(NWIN + 1, np.int64)
    wstart[1:] = np.cumsum(nblk)
    NB = int(wstart[-1])
    EP = NB * 128

    key = core * NWIN + wloc
    kcounts = np.bincount(key, minlength=P * NWIN)
    kstarts = np.zeros(P * NWIN, np.int64)
    kstarts[1:] = np.cumsum(kcounts)[:-1]
    ko = np.argsort(key, kind="stable")
    pos = np.empty(len(so), np.int64)
    pos[ko] = np.arange(len(so)) - kstarts[key[ko]]
    slot = wstart[wloc] * 128 + pos
    fi = core * EP + slot

    flat_ea = np.zeros((P * EP, ED), np.float16)
    flat_dl = np.full(P * EP, -1.0, np.float16)
    flat_si = np.zeros(P * EP, np.int64)
    flat_ea[fi] = eao
    flat_dl[fi] = (do - core * NLOC - wloc * 128).astype(np.float16)
    sc = so // NLOC
    flat_si[fi] = sc * NPL + (so - sc * NLOC)

    ea_l, dl_l, si_l = [], [], []
    for m in range(P):
        ea_l.append(np.ascontiguousarray(flat_ea[m * EP:(m + 1) * EP].T))
        dl_l.append(np.ascontiguousarray(
            flat_dl[m * EP:(m + 1) * EP].reshape(NB, 128).T))
        s16 = flat_si[m * EP:(m + 1) * EP].astype(np.int16)
        si_l.append(np.ascontiguousarray(s16.reshape(EP // 16, 16).T))
    return [int(v) for v in nblk], [int(v) for v in wstart], ea_l, dl_l, si_l


def _graph_segments(batch):
    gcnt = np.bincount(batch, minlength=G)
    assert (gcnt > 0).all(), "empty graph segment"
    gstart = np.zeros(G + 1, np.int64)
    gstart[1:] = np.cumsum(gcnt)
    parts = []  # per graph: list of (rank, lo, ln)
    for g in range(G):
        s, e = int(gstart[g]), int(gstart[g + 1])
        pl = []
        for r in range(s // NLOC, (e - 1) // NLOC + 1):
            lo = max(s, r * NLOC) - r * NLOC
            hi = min(e, (r + 1) * NLOC) - r * NLOC
            pl.append((r, lo, hi - lo))
        assert 1 <= len(pl) <= 2
        parts.append(pl)
    return parts, gcnt


def _sections(NB):
    """Packed-tensor layout: name -> (offset, nelem), 128-elem aligned."""
    EP = NB * 128
    sizes = [
        ("x", F_IN * NPL),
        ("ea", ED * EP),
        ("dl", 128 * NB),
        ("si", EP),            # [16, EP/16] int16
        ("qi", NPL),           # [16, NPL/16] int16
        ("w1", F_IN * 4 * HC),
        ("w2", HC * 4 * HC),
        ("w3", HC * 4 * HC),
        ("b", 3 * 4 * HC),
        ("ew", ED * 3 * HC),
        ("bnp", 128 * 12),
        ("rc", 128),           # per-graph 1/count row (G used)
        ("iota", 128),         # row 0..127
    ]
    off, out = 0, {}
    for name, ne in sizes:
        out[name] = (off, ne)
        off += (ne + 127) // 128 * 128
    return out, off


# -------------------------------------------------------------- bass program
def _build_program(nblk, wstart, NB, graph_parts):
    STOP = os.environ.get("BASS_GNN_STOP", "full")

    def _stopped(tag):
        order = ["l0proj", "l0edge", "l0post", "l1proj", "l1edge", "l1post",
                 "l2proj", "l2edge", "l2post", "pool", "full"]
        return order.index(STOP) < order.index(tag)

    EP = NB * 128
    SEC, TOT = _sections(NB)
    nc = bacc.Bacc("TRN2", debug=False, num_devices=P)

    PK = nc.dram_tensor("PK", [1, TOT], F16, kind="ExternalInput")
    hout = nc.dram_tensor("hout", [128, 3 * 2 * G], F32, kind="ExternalOutput")
    PKt = PK[:].tensor

    def sec2d(name, p, f, sub_off=0):
        off, ne = SEC[name]
        assert p * f + sub_off <= ne
        return bass.AP(tensor=PKt, offset=off + sub_off, ap=[[f, p], [1, f]])

    cc1 = nc.dram_tensor("cc1", [F_IN, NPL], F16, kind="Internal")
    xg1 = nc.dram_tensor("xg1", [P, F_IN, NPL], F16, kind="Internal",
                         addr_space="Shared")
    xg = [nc.dram_tensor(f"xg{l+2}", [P, HC, NPL], F32, kind="Internal",
                         addr_space="Shared") for l in range(3)]
    qtab = [nc.dram_tensor(f"qtab{l}", [NT, HC], F32, kind="Internal")
            for l in range(3)]
    ktab = [nc.dram_tensor(f"ktab{l}", [NT, HC], F32, kind="Internal")
            for l in range(3)]
    vtab = [nc.dram_tensor(f"vtab{l}", [NT, HC], F32, kind="Internal")
            for l in range(3)]
    stab = [nc.dram_tensor(f"stab{l}", [NT, HC], F32, kind="Internal")
            for l in range(3)]
    aggT = [nc.dram_tensor(f"aggT{l}", [HC, NPL], F32, kind="Internal")
            for l in range(3)]

    AG = mybir.AluOpType
    AF = mybir.ActivationFunctionType
    groups = [list(range(P))]
    holds = {}

    with tile.TileContext(nc) as tc:
        nc.gpsimd.load_library(library_config.mlp)
        with tc.tile_pool(name="const", bufs=1) as cp:
            # iota row -> broadcast tiles, identity, tail mask
            o_iota = SEC["iota"][0]
            io16 = cp.tile([128, 128], F16, name="io16")
            nc.gpsimd.dma_start(io16[:], bass.AP(
                tensor=PKt, offset=o_iota, ap=[[0, 128], [1, 128]]))
            io32 = cp.tile([128, 128], F32, name="io32")
            nc.vector.tensor_copy(io32[:], io16[:])
            ioc16 = cp.tile([128, 1], F16, name="ioc16")
            nc.gpsimd.dma_start(ioc16[:], bass.AP(
                tensor=PKt, offset=o_iota, ap=[[1, 128], [1, 1]]))
            ioc32 = cp.tile([128, 1], F32, name="ioc32")
            nc.vector.tensor_copy(ioc32[:], ioc16[:])
            s_eye = cp.tile([128, 128], F32, name="s_eye")
            nc.vector.tensor_scalar(s_eye[:], io32[:], ioc32[:], None,
                                    AG.is_equal)
            s_MASK = cp.tile([128, 1], F32, name="s_MASK")
            nc.vector.tensor_scalar(
                s_MASK[:], ioc32[:], float(NLOC - (NWIN - 1) * 128), None,
                AG.is_lt)

            # weights -> f32 SBUF
            s_W = []
            for l, (wn, K) in enumerate([("w1", F_IN), ("w2", HC), ("w3", HC)]):
                tiles = []
                for kh in range(K // 128):
                    t16 = cp.tile([128, 4 * HC], F16, name=f"w16_{l}_{kh}",
                                  tag="w16stage")
                    nc.sync.dma_start(
                        t16[:], sec2d(wn, 128, 4 * HC,
                                      sub_off=kh * 128 * 4 * HC))
                    t = cp.tile([128, 4 * HC], F32, name=f"s_W{l}_{kh}")
                    nc.vector.tensor_copy(t[:], t16[:])
                    tiles.append(t)
                s_W.append(tiles)
            s_B = []
            o_b = SEC["b"][0]
            for l in range(3):
                t16 = cp.tile([128, 4 * HC], F16, name=f"b16_{l}",
                              tag="w16stage")
                nc.gpsimd.dma_start(t16[:], bass.AP(
                    tensor=PKt, offset=o_b + l * 4 * HC,
                    ap=[[0, 128], [1, 4 * HC]]))
                t = cp.tile([128, 4 * HC], F32, name=f"s_B{l}")
                nc.vector.tensor_copy(t[:], t16[:])
                s_B.append(t)
            s_EW = cp.tile([ED, 3 * HC], F16, name="s_EW")
            nc.sync.dma_start(s_EW[:], sec2d("ew", ED, 3 * HC))
            bnp16 = cp.tile([128, 12], F16, name="bnp16")
            nc.sync.dma_start(bnp16[:], sec2d("bnp", 128, 12))
            s_BNP = cp.tile([128, 12], F32, name="s_BNP")
            nc.vector.tensor_copy(s_BNP[:], bnp16[:])
            rc16 = cp.tile([128, G], F16, name="rc16")
            nc.gpsimd.dma_start(rc16[:], bass.AP(
                tensor=PKt, offset=SEC["rc"][0], ap=[[0, 128], [1, G]]))
            s_RC = cp.tile([128, G], F32, name="s_RC")
            nc.vector.tensor_copy(s_RC[:], rc16[:])

            dl16 = cp.tile([128, NB], F16, name="dl16")
            nc.sync.dma_start(dl16[:], sec2d("dl", 128, NB))
            s_dstl = cp.tile([128, NB], F32, name="s_dstl")
            nc.vector.tensor_copy(s_dstl[:], dl16[:])
            # int16 index sections: [16, C] on wire -> replicate to 128 rows
            C = EP // 16
            s_sidx = cp.tile([128, C], I16, name="s_sidx")
            nc.gpsimd.dma_start(s_sidx[:], bass.AP(
                tensor=PKt, offset=SEC["si"][0],
                ap=[[0, 8], [C, 16], [1, C]]).bitcast(I16))
            CQ = NPL // 16
            s_qidx = cp.tile([128, CQ], I16, name="s_qidx")
            nc.gpsimd.dma_start(s_qidx[:], bass.AP(
                tensor=PKt, offset=SEC["qi"][0],
                ap=[[0, 8], [CQ, 16], [1, CQ]]).bitcast(I16))

            # ------------------------------------------- x AllGather (layer 1)
            nc.sync.dma_start(
                cc1[:].rearrange("a b -> (a b)").unsqueeze(0),
                bass.AP(tensor=PKt, offset=SEC["x"][0],
                        ap=[[1, 1], [1, F_IN * NPL]]))
            nc.gpsimd.collective_compute(
                "AllGather", AG.bypass, replica_groups=groups,
                ins=[cc1[:]], outs=[xg1[:]])

            sbn = [None, None, None]
            tbn = [None, None, None]
            sbn3 = tbn3 = None

            # ======================================================= layers
            for l in range(3):
                if _stopped(f"l{l}proj"):
                    break
                H = 4 if l == 0 else 1
                C_h = 64 if l == 0 else HC
                inv_sqrt_c = 1.0 / math.sqrt(C_h)
                K = F_IN if l == 0 else HC
                src_g = xg1 if l == 0 else xg[l - 1]

                # ---- projections: q/k/v/s tables for all NT nodes
                with (tc.tile_pool(name=f"pj{l}", bufs=3) as wp,
                      tc.tile_pool(name=f"pjp{l}", bufs=2, space="PSUM") as pp):
                    for r in range(P):
                        for c in range(NPL // 512):
                            xts = []
                            for kh in range(K // 128):
                                if l == 0:
                                    x16 = wp.tile([128, 512], F16,
                                                  tag=f"x16_{kh}")
                                    nc.sync.dma_start(
                                        x16[:],
                                        src_g[r, kh * 128:(kh + 1) * 128,
                                              c * 512:(c + 1) * 512])
                                    xb = wp.tile([128, 512], F32,
                                                 tag=f"xb{kh}")
                                    nc.vector.tensor_copy(xb[:], x16[:])
                                    xts.append(xb)
                                else:
                                    xt = wp.tile([128, 512], F32,
                                                 tag=f"xt{kh}")
                                    nc.sync.dma_start(
                                        xt[:],
                                        src_g[r, kh * 128:(kh + 1) * 128,
                                              c * 512:(c + 1) * 512])
                                    xb = wp.tile([128, 512], F32,
                                                 tag=f"xb{kh}")
                                    nc.scalar.activation(
                                        xb[:], xt[:], AF.Identity,
                                        bias=tbn[l][:, kh:kh + 1],
                                        scale=sbn[l][:, kh:kh + 1])
                                    xts.append(xb)
                            for sub in range(4):
                                row0 = r * NPL + c * 512 + sub * 128
                                for half in range(2):
                                    ppt = pp.tile(
                                        [128, 512], F32, tag=f"pp{half}",
                                        name=f"pp{l}_{r}_{c}_{sub}_{half}")
                                    nkh = K // 128
                                    for kh in range(nkh):
                                        nc.tensor.matmul(
                                            ppt[:],
                                            xts[kh][:, sub * 128:(sub + 1) * 128],
                                            s_W[l][kh][:, half * 512:(half + 1) * 512],
                                            start=(kh == 0),
                                            stop=(kh == nkh - 1))
                                    ob = wp.tile([128, 512], F32, tag="ob")
                                    nc.vector.tensor_tensor(
                                        ob[:], ppt[:],
                                        s_B[l][:, half * 512:(half + 1) * 512],
                                        AG.add)
                                    if half == 0:
                                        nc.sync.dma_start(
                                            qtab[l][row0:row0 + 128, :],
                                            ob[:, 0:HC])
                                        nc.sync.dma_start(
                                            ktab[l][row0:row0 + 128, :],
                                            ob[:, HC:2 * HC])
                                    else:
                                        nc.sync.dma_start(
                                            vtab[l][row0:row0 + 128, :],
                                            ob[:, 0:HC])
                                        nc.sync.dma_start(
                                            stab[l][row0:row0 + 128, :],
                                            ob[:, HC:2 * HC])

                # ---- edge phase: per-window segment softmax + aggregation
                if _stopped(f"l{l}edge"):
                    break
                ewsl = s_EW[:, l * HC:(l + 1) * HC]
                with (tc.tile_pool(name=f"ed{l}", bufs=3) as wp,
                      tc.tile_pool(name=f"edp{l}", bufs=2, space="PSUM") as pe):
                    for w in range(NWIN):
                        qwg = wp.tile([128, 1, HC], F32, tag="qw")
                        nc.gpsimd.dma_gather(
                            qwg[:], qtab[l][:], s_qidx[:, w * 8:(w + 1) * 8],
                            128, 128, HC)
                        swg = wp.tile([128, 1, HC], F32, tag="sw")
                        nc.gpsimd.dma_gather(
                            swg[:], stab[l][:], s_qidx[:, w * 8:(w + 1) * 8],
                            128, 128, HC)
                        qw = qwg[:].rearrange("p a f -> p (a f)")
                        sw = swg[:].rearrange("p a f -> p (a f)")
                        nb = nblk[w]
                        pagg = (pe.tile([128, HC + H], F32, tag="agg",
                                        name=f"pagg{l}_{w}")
                                if nb > 0 else None)
                        for b in range(nb):
                            blk = wstart[w] + b
                            S = wp.tile([128, 128], F32, tag="S")
                            nc.vector.tensor_scalar(
                                S[:], io32[:], s_dstl[:, blk:blk + 1], None,
                                AG.is_equal)
                            pS = pe.tile([128, 128], F32, tag="pS")
                            nc.tensor.transpose(pS[:], S[:], s_eye[:])
                            ST = wp.tile([128, 128], F32, tag="ST")
                            nc.scalar.copy(ST[:], pS[:])
                            kg = wp.tile([128, 1, HC], F32, tag="kg")
                            nc.gpsimd.dma_gather(
                                kg[:], ktab[l][:],
                                s_sidx[:, blk * 8:(blk + 1) * 8], 128, 128, HC)
                            vg = wp.tile([128, 1, HC], F32, tag="vg")
                            nc.gpsimd.dma_gather(
                                vg[:], vtab[l][:],
                                s_sidx[:, blk * 8:(blk + 1) * 8], 128, 128, HC)
                            kg2 = kg[:].rearrange("p a f -> p (a f)")
                            vg2 = vg[:].rearrange("p a f -> p (a f)")
                            eat = wp.tile([ED, 128], F16, tag="eat")
                            nc.sync.dma_start(
                                eat[:], sec2d("ea", ED, EP)[
                                    :, blk * 128:(blk + 1) * 128])
                            pE = pe.tile([128, HC], F32, tag="pE")
                            nc.tensor.matmul(pE[:], eat[:], ewsl,
                                             start=True, stop=True)
                            pQ = pe.tile([128, HC], F32, tag="pQ")
                            nc.tensor.matmul(pQ[:], ST[:], qw,
                                             start=True, stop=True)
                            kj = wp.tile([128, HC], F32, tag="kj")
                            nc.vector.tensor_tensor(kj[:], kg2, pE[:], AG.add)
                            prod = wp.tile([128, HC], F32, tag="prod")
                            nc.vector.tensor_tensor(prod[:], pQ[:], kj[:],
                                                    AG.mult)
                            al = wp.tile([128, H], F32, tag="al")
                            nc.vector.tensor_reduce(
                                al[:],
                                prod[:].rearrange("p (h c) -> p h c", h=H),
                                mybir.AxisListType.X, AG.add)
                            ex = wp.tile([128, H], F32, tag="ex")
                            nc.scalar.activation(ex[:], al[:], AF.Exp,
                                                 scale=inv_sqrt_c)
                            vj = wp.tile([128, HC], F32, tag="vj")
                            nc.vector.tensor_tensor(vj[:], vg2, pE[:], AG.add)
                            mv = wp.tile([128, HC + H], F32, tag="mv")
                            if H == 1:
                                nc.vector.tensor_scalar_mul(mv[:, 0:HC], vj[:],
                                                            ex[:, 0:1])
                            else:
                                nc.vector.tensor_tensor(
                                    mv[:, 0:HC].rearrange(
                                        "p (h c) -> p h c", h=H),
                                    vj[:].rearrange("p (h c) -> p h c", h=H),
                                    ex[:].unsqueeze(2).broadcast_to(
                                        [128, H, C_h]),
                                    AG.mult)
                            nc.vector.tensor_copy(mv[:, HC:HC + H], ex[:])
                            nc.tensor.matmul(pagg[:, 0:HC + H], S[:], mv[:],
                                             start=(b == 0),
                                             stop=(b == nb - 1))
                        # window evacuation
                        ob2 = wp.tile([128, HC], F32, tag="ob2")
                        if nb == 0:
                            nc.vector.tensor_copy(ob2[:], sw)
                        else:
                            den = wp.tile([128, H], F32, tag="den")
                            nc.vector.tensor_scalar_add(
                                den[:], pagg[:, HC:HC + H], 1e-16)
                            rc = wp.tile([128, H], F32, tag="rc")
                            nc.vector.reciprocal(rc[:], den[:])
                            ob = wp.tile([128, HC], F32, tag="ob")
                            if H == 1:
                                nc.vector.tensor_scalar_mul(
                                    ob[:], pagg[:, 0:HC], rc[:, 0:1])
                            else:
                                nc.vector.tensor_tensor(
                                    ob[:].rearrange("p (h c) -> p h c", h=H),
                                    pagg[:, 0:HC].rearrange(
                                        "p (h c) -> p h c", h=H),
                                    rc[:].unsqueeze(2).broadcast_to(
                                        [128, H, C_h]),
                                    AG.mult)
                            nc.vector.tensor_tensor(ob2[:], ob[:], sw, AG.add)
                        if w == NWIN - 1:
                            obm = wp.tile([128, HC], F32, tag="obm")
                            nc.vector.tensor_scalar_mul(obm[:], ob2[:],
                                                        s_MASK[:, 0:1])
                        else:
                            obm = ob2
                        for fh in range(2):
                            pt = pe.tile([128, 128], F32, tag="pS",
                                         name=f"pt{l}_{w}_{fh}")
                            nc.tensor.transpose(
                                pt[:], obm[:, fh * 128:(fh + 1) * 128],
                                s_eye[:])
                            tb = wp.tile([128, 128], F32, tag="tb")
                            nc.vector.tensor_copy(tb[:], pt[:])
                            nc.sync.dma_start(
                                aggT[l][fh * 128:(fh + 1) * 128,
                                        w * 128:(w + 1) * 128], tb[:])

                # ---- AllGather pre-BN output, then global BN stats
                if _stopped(f"l{l}post"):
                    break
                nc.gpsimd.collective_compute(
                    "AllGather", AG.bypass, replica_groups=groups,
                    ins=[aggT[l][:]], outs=[xg[l][:]])

                with tc.tile_pool(name=f"st{l}", bufs=2) as wp:
                    sums = wp.tile([128, 2, P], F32, tag="sums")
                    sqs = wp.tile([128, 2, P], F32, tag="sqs")
                    for r in range(P):
                        for fh in range(2):
                            ch = wp.tile([128, NPL], F32, tag="ch")
                            nc.sync.dma_start(
                                ch[:], xg[l][r, fh * 128:(fh + 1) * 128, :])
                            nc.vector.tensor_reduce(
                                sums[:, fh:fh + 1, r:r + 1].rearrange(
                                    "p a b -> p (a b)"),
                                ch[:], mybir.AxisListType.X, AG.add)
                            scr = wp.tile([128, NPL], F32, tag="scr")
                            nc.scalar.activation(
                                scr[:], ch[:], AF.Square,
                                accum_out=sqs[:, fh:fh + 1, r:r + 1].rearrange(
                                    "p a b -> p (a b)"))
                    musum = wp.tile([128, 2], F32, tag="musum")
                    nc.vector.tensor_reduce(musum[:], sums[:],
                                            mybir.AxisListType.X, AG.add)
                    mu = cp.tile([128, 2], F32, name=f"mu{l}")
                    nc.vector.tensor_scalar_mul(mu[:], musum[:], 1.0 / N)
                    sqsum = wp.tile([128, 2], F32, tag="sqsum")
                    nc.vector.tensor_reduce(sqsum[:], sqs[:],
                                            mybir.AxisListType.X, AG.add)
                    ex2 = wp.tile([128, 2], F32, tag="ex2")
                    nc.vector.tensor_scalar_mul(ex2[:], sqsum[:], 1.0 / N)
                    m2 = wp.tile([128, 2], F32, tag="m2")
                    nc.vector.tensor_tensor(m2[:], mu[:], mu[:], AG.mult)
                    var = wp.tile([128, 2], F32, tag="var")
                    nc.vector.tensor_tensor(var[:], ex2[:], m2[:], AG.subtract)
                    vpe = wp.tile([128, 2], F32, tag="vpe")
                    nc.vector.tensor_scalar_add(vpe[:], var[:], EPS)
                    sd = wp.tile([128, 2], F32, tag="sd")
                    nc.scalar.activation(sd[:], vpe[:], AF.Sqrt)
                    rstd = wp.tile([128, 2], F32, tag="rstd")
                    nc.vector.reciprocal(rstd[:], sd[:])
                    sb_t = cp.tile([128, 2], F32, name=f"sbn{l}")
                    nc.vector.tensor_tensor(sb_t[:], rstd[:],
                                            s_BNP[:, 2 * l:2 * l + 2], AG.mult)
                    tmp = wp.tile([128, 2], F32, tag="tmp")
                    nc.vector.tensor_tensor(tmp[:], mu[:], sb_t[:], AG.mult)
                    tb_t = cp.tile([128, 2], F32, name=f"tbn{l}")
                    nc.vector.tensor_tensor(
                        tb_t[:], s_BNP[:, 6 + 2 * l:8 + 2 * l], tmp[:],
                        AG.subtract)
                    if l < 2:
                        sbn[l + 1] = sb_t
                        tbn[l + 1] = tb_t
                    else:
                        sbn3, tbn3 = sb_t, tb_t

            # =============================================== pooling (layer 3)
            with tc.tile_pool(name="pool", bufs=2) as wp:
              if not _stopped("pool"):
                padd = wp.tile([128, 2, G], F32, tag="padd")
                pmax = wp.tile([128, 2, G], F32, tag="pmax")
                for fh in range(2):
                    for r in range(P):
                        ch = wp.tile([128, NLOC], F32, tag="pch")
                        nc.sync.dma_start(
                            ch[:], xg[2][r, fh * 128:(fh + 1) * 128, 0:NLOC])
                        bnc = wp.tile([128, NLOC], F32, tag="pbn")
                        nc.scalar.activation(bnc[:], ch[:], AF.Identity,
                                             bias=tbn3[:, fh:fh + 1],
                                             scale=sbn3[:, fh:fh + 1])
                        for g, pl in enumerate(graph_parts):
                            for (pr, lo, ln) in pl:
                                if pr != r:
                                    continue
                                seg = bnc[:, lo:lo + ln]
                                if len(pl) == 1:
                                    nc.vector.tensor_reduce(
                                        padd[:, fh:fh + 1, g:g + 1].rearrange(
                                            "p a b -> p (a b)"),
                                        seg, mybir.AxisListType.X, AG.add)
                                    nc.vector.tensor_reduce(
                                        pmax[:, fh:fh + 1, g:g + 1].rearrange(
                                            "p a b -> p (a b)"),
                                        seg, mybir.AxisListType.X, AG.max)
                                else:
                                    first = (pr, lo, ln) == pl[0]
                                    sfx = "a" if first else "b"
                                    ta = wp.tile([128, 1], F32,
                                                 tag=f"t{sfx}_add",
                                                 name=f"t{sfx}a_{fh}_{g}")
                                    nc.vector.tensor_reduce(
                                        ta[:], seg, mybir.AxisListType.X,
                                        AG.add)
                                    tm = wp.tile([128, 1], F32,
                                                 tag=f"t{sfx}_max",
                                                 name=f"t{sfx}m_{fh}_{g}")
                                    nc.vector.tensor_reduce(
                                        tm[:], seg, mybir.AxisListType.X,
                                        AG.max)
                                    if first:
                                        holds[(fh, g)] = (ta, tm)
                                    else:
                                        ha, hm = holds.pop((fh, g))
                                        nc.vector.tensor_tensor(
                                            padd[:, fh:fh + 1,
                                                 g:g + 1].rearrange(
                                                "p a b -> p (a b)"),
                                            ha[:], ta[:], AG.add)
                                        nc.vector.tensor_tensor(
                                            pmax[:, fh:fh + 1,
                                                 g:g + 1].rearrange(
                                                "p a b -> p (a b)"),
                                            hm[:], tm[:], AG.max)
                pmean = wp.tile([128, 2, G], F32, tag="pmean")
                for fh in range(2):
                    nc.vector.tensor_tensor(
                        pmean[:, fh, :], padd[:, fh, :], s_RC[:], AG.mult)
                nc.sync.dma_start(
                    hout[:, 0:2 * G], padd[:].rearrange("p a g -> p (a g)"))
                nc.sync.dma_start(
                    hout[:, 2 * G:4 * G],
                    pmax[:].rearrange("p a g -> p (a g)"))
                nc.sync.dma_start(
                    hout[:, 4 * G:6 * G],
                    pmean[:].rearrange("p a g -> p (a g)"))
    nc.finalize()
    return nc


# ---------------------------------------------------------------- jit runner
class _Runner:
    """Build the PJRT executable once; each call = H2D + execute + D2H."""

    def __init__(self, nc, n_cores):
        from concourse.bass2jax import (install_neuronx_cc_hook, _bass_exec_p,
                                        partition_id_tensor)
        install_neuronx_cc_hook()
        self.nc = nc
        partition_name = (nc.partition_id_tensor.name
                          if nc.partition_id_tensor else None)
        in_names, out_names, out_avals, zero_shapes = [], [], [], []
        for alloc in nc.m.functions[0].allocations:
            if not isinstance(alloc, mybir.MemoryLocationSet):
                continue
            name = alloc.memorylocations[0].name
            if alloc.kind == "ExternalInput":
                if name != partition_name:
                    in_names.append(name)
            elif alloc.kind == "ExternalOutput":
                out_names.append(name)
                shape = tuple(alloc.tensor_shape)
                dtype = mybir.dt.np(alloc.dtype)
                out_avals.append(jax.core.ShapedArray(shape, dtype))
                zero_shapes.append((shape, dtype))
        self.in_names, self.out_names = in_names, out_names
        self.zero_shapes = zero_shapes
        n_params, n_outs = len(in_names), len(out_avals)
        all_names = (list(in_names) + list(out_names)
                     + ([partition_name] if partition_name else []))

        def _body(*args):
            operands = list(args)
            if partition_name is not None:
                operands.append(partition_id_tensor())
            outs = _bass_exec_p.bind(
                *operands, out_avals=tuple(out_avals),
                in_names=tuple(all_names), out_names=tuple(out_names),
                lowering_input_output_aliases=(), sim_require_finite=True,
                sim_require_nnan=True, nc=nc)
            return tuple(outs)

        devices = jax.devices()[:n_cores]
        mesh = Mesh(np.asarray(devices), ("core",))
        in_specs = (PartitionSpec("core"),) * (n_params + n_outs)
        out_specs = (PartitionSpec("core"),) * n_outs
        self.n_cores = n_cores
        self.fn = jax.jit(
            shard_map(_body, mesh=mesh, in_specs=in_specs,
                      out_specs=out_specs, check_rep=False),
            donate_argnums=tuple(range(n_params, n_params + n_outs)),
            keep_unused=True)

    def __call__(self, in_maps):
        per_core = [[np.asarray(m[n]) for n in self.in_names] for m in in_maps]
        concat_in = [np.concatenate(
            [per_core[c][i] for c in range(self.n_cores)], axis=0)
            for i in range(len(self.in_names))]
        zeros = [np.zeros((self.n_cores * s[0], *s[1:]), d)
                 for s, d in self.zero_shapes]
        outs = self.fn(*concat_in, *zeros)
        outs = [np.asarray(o) for o in outs]
        return [{n: outs[i].reshape(self.n_cores, *self.zero_shapes[i][0])[c]
                 for i, n in enumerate(self.out_names)}
                for c in range(self.n_cores)]


# --------------------------------------------------------------------- kernel
def kernel(x, edge_index, edge_attr, batch,
           q1w, q1b, k1w, k1b, v1w, v1b, e1w, s1w, s1b, bn1w, bn1b,
           q2w, q2b, k2w, k2b, v2w, v2b, e2w, s2w, s2b, bn2w, bn2b,
           q3w, q3b, k3w, k3b, v3w, v3b, e3w, s3w, s3b, bn3w, bn3b,
           m1w, m1b, pa, m2w, m2b):
    global LAST_EXEC_NS
    x = np.asarray(x, np.float32)
    edge_index = np.asarray(edge_index)
    edge_attr = np.asarray(edge_attr, np.float32)
    batch = np.asarray(batch)
    src, dst = edge_index[0], edge_index[1]

    nblk, wstart, ea_l, dl_l, si_l = _pack_edges(src, dst, edge_attr)
    NB = wstart[-1]
    graph_parts, gcnt = _graph_segments(batch)
    SEC, TOT = _sections(NB)

    key = (tuple(nblk), tuple(tuple(p) for pl in graph_parts for p in pl))
    if key in _CACHE:
        runner = _CACHE[key]
    else:
        nc = _build_program(nblk, wstart, NB, graph_parts)
        runner = _Runner(nc, P)
        _CACHE[key] = runner

    xp = np.zeros((NT, F_IN), np.float16)
    for m in range(P):
        xp[m * NPL:m * NPL + NLOC] = x[m * NLOC:(m + 1) * NLOC]

    def f16(a):
        return np.asarray(a, np.float16)

    com = {}   # replicated sections, flat f16
    com["w1"] = f16(np.hstack([q1w, k1w, v1w, s1w])).ravel()
    com["w2"] = f16(np.hstack([q2w, k2w, v2w, s2w])).ravel()
    com["w3"] = f16(np.hstack([q3w, k3w, v3w, s3w])).ravel()
    com["b"] = f16(np.concatenate(
        [np.hstack([q1b, k1b, v1b, s1b]), np.hstack([q2b, k2b, v2b, s2b]),
         np.hstack([q3b, k3b, v3b, s3b])]))
    com["ew"] = f16(np.hstack([e1w, e2w, e3w])).ravel()
    bnp = np.zeros((128, 12), np.float16)
    for l, (bw, bb) in enumerate([(bn1w, bn1b), (bn2w, bn2b), (bn3w, bn3b)]):
        bnp[:, 2 * l:2 * l + 2] = np.asarray(bw).reshape(2, 128).T
        bnp[:, 6 + 2 * l:8 + 2 * l] = np.asarray(bb).reshape(2, 128).T
    com["bnp"] = bnp.ravel()
    rcv = np.zeros(128, np.float16)
    rcv[0:G] = (1.0 / np.maximum(gcnt, 1)).astype(np.float16)
    com["rc"] = rcv
    com["iota"] = np.arange(128, dtype=np.float16)

    in_maps = []
    for m in range(P):
        pk = np.zeros(TOT, np.float16)
        o, ne = SEC["x"]
        pk[o:o + ne] = np.ascontiguousarray(
            xp[m * NPL:(m + 1) * NPL].T).ravel()
        o, ne = SEC["ea"]
        pk[o:o + ne] = ea_l[m].ravel()
        o, ne = SEC["dl"]
        pk[o:o + ne] = dl_l[m].ravel()
        o, ne = SEC["si"]
        pk[o:o + ne] = si_l[m].ravel().view(np.float16)
        o, ne = SEC["qi"]
        ids = (m * NPL + np.arange(NPL)).astype(np.int16)
        pk[o:o + ne] = np.ascontiguousarray(
            ids.reshape(NPL // 16, 16).T).ravel().view(np.float16)
        for name, arr in com.items():
            o, ne = SEC[name]
            pk[o:o + len(arr)] = arr
        in_maps.append({"PK": pk[None, :]})

    res = runner(in_maps)
    if os.environ.get("BASS_GNN_TIME") == "1":
        t0 = time.perf_counter_ns()
        res = runner(in_maps)
        LAST_EXEC_NS = time.perf_counter_ns() - t0

    ho = np.asarray(res[0]["hout"], np.float32)   # [128, 384]
    x_add = np.empty((G, HC), np.float32)
    x_max = np.empty((G, HC), np.float32)
    x_mean = np.empty((G, HC), np.float32)
    for i, arr in enumerate([x_add, x_max, x_mean]):
        blk = ho[:, i * 2 * G:(i + 1) * 2 * G].reshape(128, 2, G)
        arr[:, 0:128] = blk[:, 0, :].T
        arr[:, 128:256] = blk[:, 1, :].T

    h = np.concatenate([x_add, x_max, x_mean], axis=1).astype(np.float32)
    h = h @ np.asarray(m1w, np.float32) + np.asarray(m1b, np.float32)
    h = np.where(h >= 0, h, np.float32(pa) * h)
    lg = h @ np.asarray(m2w, np.float32) + np.asarray(m2b, np.float32)
    mx = lg.max(axis=1, keepdims=True)
    sh = lg - mx
    return (sh - np.log(np.exp(sh).sum(axis=1, keepdims=True))).astype(np.float32)


# revision 18
# speedup vs baseline: 7.4329x; 1.3487x over previous
"""Full 3-layer TransformerConv GNN on 8 Trainium2 cores.

Sharding: edges sorted by dst and partitioned into 8 contiguous dst-node
ranges (2500 nodes/core, padded to 2560).  Node projections (q/k/v/skip)
are computed replicated on every core into global DRAM tables; each core
runs segment-softmax message aggregation only for its 20 local 128-node
dst windows via one-hot scatter matmuls (PSUM-accumulated per window).
Pre-BN layer outputs are AllGathered (feat-major) between layers; BN
statistics are computed replicated from the gathered tensor.  Per-graph
sum/max/mean pooling happens on device; only the [64,768] pooled tensor
returns to the host, which applies the tiny MLP head.

All host->device payload travels in ONE packed fp16 tensor per core
(int16 index sections bitcast) to minimize axon-tunnel transfer time,
which dominates the dispatch wall clock.  Device compute stays fp32.

Self-contained: shapes hardcoded, sharding derived from the inputs.
"""
import math
import os
import time
import numpy as np

import jax
from jax.sharding import Mesh, PartitionSpec
from jax.experimental.shard_map import shard_map

from concourse import bacc, bass, tile, mybir, library_config

P = 8
N, E, F_IN, ED, G = 20000, 640000, 128, 4, 64
HC = 256
NLOC = N // P          # 2500
NWIN = 20              # 128-node dst windows per core
NPL = NWIN * 128       # 2560 padded local nodes
NT = P * NPL           # 20480 padded global nodes
EPS = 1e-5
F32 = mybir.dt.float32
F16 = mybir.dt.float16
I16 = mybir.dt.int16

LAST_EXEC_NS = None

_CACHE = {}


# ----------------------------------------------------------------- host pack
def _pack_edges(src, dst, edge_attr):
    order = np.argsort(dst, kind="stable")
    so = src[order].astype(np.int64)
    do = dst[order].astype(np.int64)
    eao = edge_attr[order].astype(np.float32)

    core = do // NLOC
    wloc = (do - core * NLOC) >> 7          # local window [0, NWIN)
    cw = np.zeros((P, NWIN), np.int64)
    np.add.at(cw, (core, wloc), 1)
    nblk = (-(-cw // 128)).max(axis=0)      # common blocks per window
    wstart = np.zeros(NWIN + 1, np.int64)
    wstart[1:] = np.cumsum(nblk)
    NB = int(wstart[-1])
    EP = NB * 128

    key = core * NWIN + wloc
    kcounts = np.bincount(key, minlength=P * NWIN)
    kstarts = np.zeros(P * NWIN, np.int64)
    kstarts[1:] = np.cumsum(kcounts)[:-1]
    ko = np.argsort(key, kind="stable")
    pos = np.empty(len(so), np.int64)
    pos[ko] = np.arange(len(so)) - kstarts[key[ko]]
    slot = wstart[wloc] * 128 + pos
    fi = core * EP + slot

    flat_ea = np.zeros((P * EP, ED), np.float16)
    flat_dl = np.full(P * EP, -1.0, np.float16)
    flat_si = np.zeros(P * EP, np.int64)
    flat_ea[fi] = eao
    flat_dl[fi] = (do - core * NLOC - wloc * 128).astype(np.float16)
    sc = so // NLOC
    flat_si[fi] = sc * NPL + (so - sc * NLOC)

    ea_l, dl_l, si_l = [], [], []
    for m in range(P):
        ea_l.append(np.ascontiguousarray(flat_ea[m * EP:(m + 1) * EP].T))
        dl_l.append(np.ascontiguousarray(
            flat_dl[m * EP:(m + 1) * EP].reshape(NB, 128).T))
        s16 = flat_si[m * EP:(m + 1) * EP].astype(np.int16)
        si_l.append(np.ascontiguousarray(s16.reshape(EP // 16, 16).T))
    return [int(v) for v in nblk], [int(v) for v in wstart], ea_l, dl_l, si_l


def _graph_segments(batch):
    gcnt = np.bincount(batch, minlength=G)
    assert (gcnt > 0).all(), "empty graph segment"
    gstart = np.zeros(G + 1, np.int64)
    gstart[1:] = np.cumsum(gcnt)
    parts = []  # per graph: list of (rank, lo, ln)
    for g in range(G):
        s, e = int(gstart[g]), int(gstart[g + 1])
        pl = []
        for r in range(s // NLOC, (e - 1) // NLOC + 1):
            lo = max(s, r * NLOC) - r * NLOC
            hi = min(e, (r + 1) * NLOC) - r * NLOC
            pl.append((r, lo, hi - lo))
        assert 1 <= len(pl) <= 2
        parts.append(pl)
    return parts, gcnt


def _sections(NB):
    """Packed-tensor layout: name -> (offset, nelem), 128-elem aligned."""
    EP = NB * 128
    sizes = [
        ("x", F_IN * NPL),
        ("ea", ED * EP),
        ("dl", 128 * NB),
        ("si", EP),            # [16, EP/16] int16
        ("qi", NPL),           # [16, NPL/16] int16
        ("w1", F_IN * 4 * HC),
        ("w2", HC * 4 * HC),
        ("w3", HC * 4 * HC),
        ("b", 3 * 4 * HC),
        ("ew", ED * 3 * HC),
        ("bnp", 128 * 12),
        ("rc", 128),           # per-graph 1/count row (G used)
        ("iota", 128),         # row 0..127
    ]
    off, out = 0, {}
    for name, ne in sizes:
        out[name] = (off, ne)
        off += (ne + 127) // 128 * 128
    return out, off


# -------------------------------------------------------------- bass program
def _build_program(nblk, wstart, NB, graph_parts):
    STOP = os.environ.get("BASS_GNN_STOP", "full")

    def _stopped(tag):
        order = ["l0proj", "l0edge", "l0post", "l1proj", "l1edge", "l1post",
                 "l2proj", "l2edge", "l2post", "pool", "full"]
        return order.index(STOP) < order.index(tag)

    EP = NB * 128
    SEC, TOT = _sections(NB)
    nc = bacc.Bacc("TRN2", debug=False, num_devices=P)

    PK = nc.dram_tensor("PK", [1, TOT], F16, kind="ExternalInput")
    hout = nc.dram_tensor("hout", [128, 3 * 2 * G], F16, kind="ExternalOutput")
    PKt = PK[:].tensor

    WREP = ("w1", "w2", "w3", "b", "ew", "bnp", "rc", "iota")

    def sec2d(name, p, f, sub_off=0):
        off, ne = SEC[name]
        assert p * f + sub_off <= ne
        if name in WREP:
            return bass.AP(tensor=wbuf[:].tensor, offset=off - SEC["w1"][0] + sub_off,
                           ap=[[f, p], [1, f]])
        return bass.AP(tensor=PKt, offset=off + sub_off, ap=[[f, p], [1, f]])

    def wsec_off(name):
        return SEC[name][0] - SEC["w1"][0]

    W0 = SEC["w1"][0]
    WSPAN = TOT - W0
    wbuf = nc.dram_tensor("wbuf", [1, WSPAN], F16, kind="Internal")
    cc1 = nc.dram_tensor("cc1", [F_IN, NPL], F16, kind="Internal")
    xg1 = nc.dram_tensor("xg1", [P, F_IN, NPL], F16, kind="Internal",
                         addr_space="Shared")
    xg = [nc.dram_tensor(f"xg{l+2}", [P, HC, NPL], F32, kind="Internal",
                         addr_space="Shared") for l in range(3)]
    qtab = [nc.dram_tensor(f"qtab{l}", [NT, HC], F32, kind="Internal")
            for l in range(3)]
    ktab = [nc.dram_tensor(f"ktab{l}", [NT, HC], F32, kind="Internal")
            for l in range(3)]
    vtab = [nc.dram_tensor(f"vtab{l}", [NT, HC], F32, kind="Internal")
            for l in range(3)]
    stab = [nc.dram_tensor(f"stab{l}", [NT, HC], F32, kind="Internal")
            for l in range(3)]
    aggT = [nc.dram_tensor(f"aggT{l}", [HC, NPL], F32, kind="Internal")
            for l in range(3)]

    AG = mybir.AluOpType
    AF = mybir.ActivationFunctionType
    groups = [list(range(P))]
    holds = {}

    with tile.TileContext(nc) as tc:
        nc.gpsimd.load_library(library_config.mlp)
        nc.sync.dma_start(
            wbuf[:],
            bass.AP(tensor=PKt, offset=W0, ap=[[1, 1], [1, WSPAN]]))
        nc.gpsimd.collective_compute(
            "AllReduce", mybir.AluOpType.add, replica_groups=groups,
            ins=[wbuf[:]], outs=[wbuf[:]])
        with tc.tile_pool(name="const", bufs=1) as cp:
            # iota row -> broadcast tiles, identity, tail mask
            o_iota = wsec_off("iota")
            io16 = cp.tile([128, 128], F16, name="io16")
            nc.gpsimd.dma_start(io16[:], bass.AP(
                tensor=wbuf[:].tensor, offset=o_iota, ap=[[0, 128], [1, 128]]))
            io32 = cp.tile([128, 128], F32, name="io32")
            nc.vector.tensor_copy(io32[:], io16[:])
            ioc16 = cp.tile([128, 1], F16, name="ioc16")
            nc.gpsimd.dma_start(ioc16[:], bass.AP(
                tensor=wbuf[:].tensor, offset=o_iota, ap=[[1, 128], [1, 1]]))
            ioc32 = cp.tile([128, 1], F32, name="ioc32")
            nc.vector.tensor_copy(ioc32[:], ioc16[:])
            s_eye = cp.tile([128, 128], F32, name="s_eye")
            nc.vector.tensor_scalar(s_eye[:], io32[:], ioc32[:], None,
                                    AG.is_equal)
            s_MASK = cp.tile([128, 1], F32, name="s_MASK")
            nc.vector.tensor_scalar(
                s_MASK[:], ioc32[:], float(NLOC - (NWIN - 1) * 128), None,
                AG.is_lt)

            # weights -> f32 SBUF
            s_W = []
            for l, (wn, K) in enumerate([("w1", F_IN), ("w2", HC), ("w3", HC)]):
                tiles = []
                for kh in range(K // 128):
                    t16 = cp.tile([128, 4 * HC], F16, name=f"w16_{l}_{kh}",
                                  tag="w16stage")
                    nc.sync.dma_start(
                        t16[:], sec2d(wn, 128, 4 * HC,
                                      sub_off=kh * 128 * 4 * HC))
                    t = cp.tile([128, 4 * HC], F32, name=f"s_W{l}_{kh}")
                    nc.vector.tensor_copy(t[:], t16[:])
                    tiles.append(t)
                s_W.append(tiles)
            s_B = []
            o_b = wsec_off("b")
            for l in range(3):
                t16 = cp.tile([128, 4 * HC], F16, name=f"b16_{l}",
                              tag="w16stage")
                nc.gpsimd.dma_start(t16[:], bass.AP(
                    tensor=wbuf[:].tensor, offset=o_b + l * 4 * HC,
                    ap=[[0, 128], [1, 4 * HC]]))
                t = cp.tile([128, 4 * HC], F32, name=f"s_B{l}")
                nc.vector.tensor_copy(t[:], t16[:])
                s_B.append(t)
            s_EW = cp.tile([ED, 3 * HC], F16, name="s_EW")
            nc.sync.dma_start(s_EW[:], sec2d("ew", ED, 3 * HC))
            bnp16 = cp.tile([128, 12], F16, name="bnp16")
            nc.sync.dma_start(bnp16[:], sec2d("bnp", 128, 12))
            s_BNP = cp.tile([128, 12], F32, name="s_BNP")
            nc.vector.tensor_copy(s_BNP[:], bnp16[:])
            rc16 = cp.tile([128, G], F16, name="rc16")
            nc.gpsimd.dma_start(rc16[:], bass.AP(
                tensor=wbuf[:].tensor, offset=wsec_off("rc"),
                ap=[[0, 128], [1, G]]))
            s_RC = cp.tile([128, G], F32, name="s_RC")
            nc.vector.tensor_copy(s_RC[:], rc16[:])

            dl16 = cp.tile([128, NB], F16, name="dl16")
            nc.sync.dma_start(dl16[:], sec2d("dl", 128, NB))
            s_dstl = cp.tile([128, NB], F32, name="s_dstl")
            nc.vector.tensor_copy(s_dstl[:], dl16[:])
            # int16 index sections: [16, C] on wire -> replicate to 128 rows
            C = EP // 16
            s_sidx = cp.tile([128, C], I16, name="s_sidx")
            nc.gpsimd.dma_start(s_sidx[:], bass.AP(
                tensor=PKt, offset=SEC["si"][0],
                ap=[[0, 8], [C, 16], [1, C]]).bitcast(I16))
            CQ = NPL // 16
            s_qidx = cp.tile([128, CQ], I16, name="s_qidx")
            nc.gpsimd.dma_start(s_qidx[:], bass.AP(
                tensor=PKt, offset=SEC["qi"][0],
                ap=[[0, 8], [CQ, 16], [1, CQ]]).bitcast(I16))

            # ------------------------------------------- x AllGather (layer 1)
            nc.sync.dma_start(
                cc1[:].rearrange("a b -> (a b)").unsqueeze(0),
                bass.AP(tensor=PKt, offset=SEC["x"][0],
                        ap=[[1, 1], [1, F_IN * NPL]]))
            nc.gpsimd.collective_compute(
                "AllGather", AG.bypass, replica_groups=groups,
                ins=[cc1[:]], outs=[xg1[:]])

            sbn = [None, None, None]
            tbn = [None, None, None]
            sbn3 = tbn3 = None

            # ======================================================= layers
            for l in range(3):
                if _stopped(f"l{l}proj"):
                    break
                H = 4 if l == 0 else 1
                C_h = 64 if l == 0 else HC
                inv_sqrt_c = 1.0 / math.sqrt(C_h)
                K = F_IN if l == 0 else HC
                src_g = xg1 if l == 0 else xg[l - 1]

                # ---- projections: q/k/v/s tables for all NT nodes
                with (tc.tile_pool(name=f"pj{l}", bufs=3) as wp,
                      tc.tile_pool(name=f"pjp{l}", bufs=2, space="PSUM") as pp):
                    for r in range(P):
                        for c in range(NPL // 512):
                            xts = []
                            for kh in range(K // 128):
                                if l == 0:
                                    x16 = wp.tile([128, 512], F16,
                                                  tag=f"x16_{kh}")
                                    nc.sync.dma_start(
                                        x16[:],
                                        src_g[r, kh * 128:(kh + 1) * 128,
                                              c * 512:(c + 1) * 512])
                                    xb = wp.tile([128, 512], F32,
                                                 tag=f"xb{kh}")
                                    nc.vector.tensor_copy(xb[:], x16[:])
                                    xts.append(xb)
                                else:
                                    xt = wp.tile([128, 512], F32,
                                                 tag=f"xt{kh}")
                                    nc.sync.dma_start(
                                        xt[:],
                                        src_g[r, kh * 128:(kh + 1) * 128,
                                              c * 512:(c + 1) * 512])
                                    xb = wp.tile([128, 512], F32,
                                                 tag=f"xb{kh}")
                                    nc.scalar.activation(
                                        xb[:], xt[:], AF.Identity,
                                        bias=tbn[l][:, kh:kh + 1],
                                        scale=sbn[l][:, kh:kh + 1])
                                    xts.append(xb)
                            for sub in range(4):
                                row0 = r * NPL + c * 512 + sub * 128
                                for half in range(2):
                                    ppt = pp.tile(
                                        [128, 512], F32, tag=f"pp{half}",
                                        name=f"pp{l}_{r}_{c}_{sub}_{half}")
                                    nkh = K // 128
                                    for kh in range(nkh):
                                        nc.tensor.matmul(
                                            ppt[:],
                                            xts[kh][:, sub * 128:(sub + 1) * 128],
                                            s_W[l][kh][:, half * 512:(half + 1) * 512],
                                            start=(kh == 0),
                                            stop=(kh == nkh - 1))
                                    ob = wp.tile([128, 512], F32, tag="ob")
                                    nc.vector.tensor_tensor(
                                        ob[:], ppt[:],
                                        s_B[l][:, half * 512:(half + 1) * 512],
                                        AG.add)
                                    if half == 0:
                                        nc.sync.dma_start(
                                            qtab[l][row0:row0 + 128, :],
                                            ob[:, 0:HC])
                                        nc.sync.dma_start(
                                            ktab[l][row0:row0 + 128, :],
                                            ob[:, HC:2 * HC])
                                    else:
                                        nc.sync.dma_start(
                                            vtab[l][row0:row0 + 128, :],
                                            ob[:, 0:HC])
                                        nc.sync.dma_start(
                                            stab[l][row0:row0 + 128, :],
                                            ob[:, HC:2 * HC])

                # ---- edge phase: per-window segment softmax + aggregation
                if _stopped(f"l{l}edge"):
                    break
                ewsl = s_EW[:, l * HC:(l + 1) * HC]
                with (tc.tile_pool(name=f"ed{l}", bufs=3) as wp,
                      tc.tile_pool(name=f"edp{l}", bufs=2, space="PSUM") as pe):
                    for w in range(NWIN):
                        qwg = wp.tile([128, 1, HC], F32, tag="qw")
                        nc.gpsimd.dma_gather(
                            qwg[:], qtab[l][:], s_qidx[:, w * 8:(w + 1) * 8],
                            128, 128, HC)
                        swg = wp.tile([128, 1, HC], F32, tag="sw")
                        nc.gpsimd.dma_gather(
                            swg[:], stab[l][:], s_qidx[:, w * 8:(w + 1) * 8],
                            128, 128, HC)
                        qw = qwg[:].rearrange("p a f -> p (a f)")
                        sw = swg[:].rearrange("p a f -> p (a f)")
                        nb = nblk[w]
                        pagg = (pe.tile([128, HC + H], F32, tag="agg",
                                        name=f"pagg{l}_{w}")
                                if nb > 0 else None)
                        for b in range(nb):
                            blk = wstart[w] + b
                            S = wp.tile([128, 128], F32, tag="S")
                            nc.vector.tensor_scalar(
                                S[:], io32[:], s_dstl[:, blk:blk + 1], None,
                                AG.is_equal)
                            pS = pe.tile([128, 128], F32, tag="pS")
                            nc.tensor.transpose(pS[:], S[:], s_eye[:])
                            ST = wp.tile([128, 128], F32, tag="ST")
                            nc.scalar.copy(ST[:], pS[:])
                            kg = wp.tile([128, 1, HC], F32, tag="kg")
                            nc.gpsimd.dma_gather(
                                kg[:], ktab[l][:],
                                s_sidx[:, blk * 8:(blk + 1) * 8], 128, 128, HC)
                            vg = wp.tile([128, 1, HC], F32, tag="vg")
                            nc.gpsimd.dma_gather(
                                vg[:], vtab[l][:],
                                s_sidx[:, blk * 8:(blk + 1) * 8], 128, 128, HC)
                            kg2 = kg[:].rearrange("p a f -> p (a f)")
                            vg2 = vg[:].rearrange("p a f -> p (a f)")
                            eat = wp.tile([ED, 128], F16, tag="eat")
                            nc.sync.dma_start(
                                eat[:], sec2d("ea", ED, EP)[
                                    :, blk * 128:(blk + 1) * 128])
                            pE = pe.tile([128, HC], F32, tag="pE")
                            nc.tensor.matmul(pE[:], eat[:], ewsl,
                                             start=True, stop=True)
                            pQ = pe.tile([128, HC], F32, tag="pQ")
                            nc.tensor.matmul(pQ[:], ST[:], qw,
                                             start=True, stop=True)
                            kj = wp.tile([128, HC], F32, tag="kj")
                            nc.vector.tensor_tensor(kj[:], kg2, pE[:], AG.add)
                            prod = wp.tile([128, HC], F32, tag="prod")
                            nc.vector.tensor_tensor(prod[:], pQ[:], kj[:],
                                                    AG.mult)
                            al = wp.tile([128, H], F32, tag="al")
                            nc.vector.tensor_reduce(
                                al[:],
                                prod[:].rearrange("p (h c) -> p h c", h=H),
                                mybir.AxisListType.X, AG.add)
                            ex = wp.tile([128, H], F32, tag="ex")
                            nc.scalar.activation(ex[:], al[:], AF.Exp,
                                                 scale=inv_sqrt_c)
                            vj = wp.tile([128, HC], F32, tag="vj")
                            nc.vector.tensor_tensor(vj[:], vg2, pE[:], AG.add)
                            mv = wp.tile([128, HC + H], F32, tag="mv")
                            if H == 1:
                                nc.vector.tensor_scalar_mul(mv[:, 0:HC], vj[:],
                                                            ex[:, 0:1])
                            else:
                                nc.vector.tensor_tensor(
                                    mv[:, 0:HC].rearrange(
                                        "p (h c) -> p h c", h=H),
                                    vj[:].rearrange("p (h c) -> p h c", h=H),
                                    ex[:].unsqueeze(2).broadcast_to(
                                        [128, H, C_h]),
                                    AG.mult)
                            nc.vector.tensor_copy(mv[:, HC:HC + H], ex[:])
                            nc.tensor.matmul(pagg[:, 0:HC + H], S[:], mv[:],
                                             start=(b == 0),
                                             stop=(b == nb - 1))
                        # window evacuation
                        ob2 = wp.tile([128, HC], F32, tag="ob2")
                        if nb == 0:
                            nc.vector.tensor_copy(ob2[:], sw)
                        else:
                            den = wp.tile([128, H], F32, tag="den")
                            nc.vector.tensor_scalar_add(
                                den[:], pagg[:, HC:HC + H], 1e-16)
                            rc = wp.tile([128, H], F32, tag="rc")
                            nc.vector.reciprocal(rc[:], den[:])
                            ob = wp.tile([128, HC], F32, tag="ob")
                            if H == 1:
                                nc.vector.tensor_scalar_mul(
                                    ob[:], pagg[:, 0:HC], rc[:, 0:1])
                            else:
                                nc.vector.tensor_tensor(
                                    ob[:].rearrange("p (h c) -> p h c", h=H),
                                    pagg[:, 0:HC].rearrange(
                                        "p (h c) -> p h c", h=H),
                                    rc[:].unsqueeze(2).broadcast_to(
                                        [128, H, C_h]),
                                    AG.mult)
                            nc.vector.tensor_tensor(ob2[:], ob[:], sw, AG.add)
                        if w == NWIN - 1:
                            obm = wp.tile([128, HC], F32, tag="obm")
                            nc.vector.tensor_scalar_mul(obm[:], ob2[:],
                                                        s_MASK[:, 0:1])
                        else:
                            obm = ob2
                        for fh in range(2):
                            pt = pe.tile([128, 128], F32, tag="pS",
                                         name=f"pt{l}_{w}_{fh}")
                            nc.tensor.transpose(
                                pt[:], obm[:, fh * 128:(fh + 1) * 128],
                                s_eye[:])
                            tb = wp.tile([128, 128], F32, tag="tb")
                            nc.vector.tensor_copy(tb[:], pt[:])
                            nc.sync.dma_start(
                                aggT[l][fh * 128:(fh + 1) * 128,
                                        w * 128:(w + 1) * 128], tb[:])

                # ---- AllGather pre-BN output, then global BN stats
                if _stopped(f"l{l}post"):
                    break
                nc.gpsimd.collective_compute(
                    "AllGather", AG.bypass, replica_groups=groups,
                    ins=[aggT[l][:]], outs=[xg[l][:]])

                with tc.tile_pool(name=f"st{l}", bufs=2) as wp:
                    sums = wp.tile([128, 2, P], F32, tag="sums")
                    sqs = wp.tile([128, 2, P], F32, tag="sqs")
                    for r in range(P):
                        for fh in range(2):
                            ch = wp.tile([128, NPL], F32, tag="ch")
                            nc.sync.dma_start(
                                ch[:], xg[l][r, fh * 128:(fh + 1) * 128, :])
                            nc.vector.tensor_reduce(
                                sums[:, fh:fh + 1, r:r + 1].rearrange(
                                    "p a b -> p (a b)"),
                                ch[:], mybir.AxisListType.X, AG.add)
                            scr = wp.tile([128, NPL], F32, tag="scr")
                            nc.scalar.activation(
                                scr[:], ch[:], AF.Square,
                                accum_out=sqs[:, fh:fh + 1, r:r + 1].rearrange(
                                    "p a b -> p (a b)"))
                    musum = wp.tile([128, 2], F32, tag="musum")
                    nc.vector.tensor_reduce(musum[:], sums[:],
                                            mybir.AxisListType.X, AG.add)
                    mu = cp.tile([128, 2], F32, name=f"mu{l}")
                    nc.vector.tensor_scalar_mul(mu[:], musum[:], 1.0 / N)
                    sqsum = wp.tile([128, 2], F32, tag="sqsum")
                    nc.vector.tensor_reduce(sqsum[:], sqs[:],
                                            mybir.AxisListType.X, AG.add)
                    ex2 = wp.tile([128, 2], F32, tag="ex2")
                    nc.vector.tensor_scalar_mul(ex2[:], sqsum[:], 1.0 / N)
                    m2 = wp.tile([128, 2], F32, tag="m2")
                    nc.vector.tensor_tensor(m2[:], mu[:], mu[:], AG.mult)
                    var = wp.tile([128, 2], F32, tag="var")
                    nc.vector.tensor_tensor(var[:], ex2[:], m2[:], AG.subtract)
                    vpe = wp.tile([128, 2], F32, tag="vpe")
                    nc.vector.tensor_scalar_add(vpe[:], var[:], EPS)
                    sd = wp.tile([128, 2], F32, tag="sd")
                    nc.scalar.activation(sd[:], vpe[:], AF.Sqrt)
                    rstd = wp.tile([128, 2], F32, tag="rstd")
                    nc.vector.reciprocal(rstd[:], sd[:])
                    sb_t = cp.tile([128, 2], F32, name=f"sbn{l}")
                    nc.vector.tensor_tensor(sb_t[:], rstd[:],
                                            s_BNP[:, 2 * l:2 * l + 2], AG.mult)
                    tmp = wp.tile([128, 2], F32, tag="tmp")
                    nc.vector.tensor_tensor(tmp[:], mu[:], sb_t[:], AG.mult)
                    tb_t = cp.tile([128, 2], F32, name=f"tbn{l}")
                    nc.vector.tensor_tensor(
                        tb_t[:], s_BNP[:, 6 + 2 * l:8 + 2 * l], tmp[:],
                        AG.subtract)
                    if l < 2:
                        sbn[l + 1] = sb_t
                        tbn[l + 1] = tb_t
                    else:
                        sbn3, tbn3 = sb_t, tb_t

            # =============================================== pooling (layer 3)
            with tc.tile_pool(name="pool", bufs=2) as wp:
              if not _stopped("pool"):
                padd = wp.tile([128, 2, G], F32, tag="padd")
                pmax = wp.tile([128, 2, G], F32, tag="pmax")
                for fh in range(2):
                    for r in range(P):
                        ch = wp.tile([128, NLOC], F32, tag="pch")
                        nc.sync.dma_start(
                            ch[:], xg[2][r, fh * 128:(fh + 1) * 128, 0:NLOC])
                        bnc = wp.tile([128, NLOC], F32, tag="pbn")
                        nc.scalar.activation(bnc[:], ch[:], AF.Identity,
                                             bias=tbn3[:, fh:fh + 1],
                                             scale=sbn3[:, fh:fh + 1])
                        for g, pl in enumerate(graph_parts):
                            for (pr, lo, ln) in pl:
                                if pr != r:
                                    continue
                                seg = bnc[:, lo:lo + ln]
                                if len(pl) == 1:
                                    nc.vector.tensor_reduce(
                                        padd[:, fh:fh + 1, g:g + 1].rearrange(
                                            "p a b -> p (a b)"),
                                        seg, mybir.AxisListType.X, AG.add)
                                    nc.vector.tensor_reduce(
                                        pmax[:, fh:fh + 1, g:g + 1].rearrange(
                                            "p a b -> p (a b)"),
                                        seg, mybir.AxisListType.X, AG.max)
                                else:
                                    first = (pr, lo, ln) == pl[0]
                                    sfx = "a" if first else "b"
                                    ta = wp.tile([128, 1], F32,
                                                 tag=f"t{sfx}_add",
                                                 name=f"t{sfx}a_{fh}_{g}")
                                    nc.vector.tensor_reduce(
                                        ta[:], seg, mybir.AxisListType.X,
                                        AG.add)
                                    tm = wp.tile([128, 1], F32,
                                                 tag=f"t{sfx}_max",
                                                 name=f"t{sfx}m_{fh}_{g}")
                                    nc.vector.tensor_reduce(
                                        tm[:], seg, mybir.AxisListType.X,
                                        AG.max)
                                    if first:
                                        holds[(fh, g)] = (ta, tm)
                                    else:
                                        ha, hm = holds.pop((fh, g))
                                        nc.vector.tensor_tensor(
                                            padd[:, fh:fh + 1,
                                                 g:g + 1].rearrange(
                                                "p a b -> p (a b)"),
                                            ha[:], ta[:], AG.add)
                                        nc.vector.tensor_tensor(
                                            pmax[:, fh:fh + 1,
                                                 g:g + 1].rearrange(
                                                "p a b -> p (a b)"),
                                            hm[:], tm[:], AG.max)
                pmean = wp.tile([128, 2, G], F32, tag="pmean")
                for fh in range(2):
                    nc.vector.tensor_tensor(
                        pmean[:, fh, :], padd[:, fh, :], s_RC[:], AG.mult)
                h16 = wp.tile([128, 3 * 2 * G], F16, tag="h16")
                nc.vector.tensor_copy(
                    h16[:, 0:2 * G], padd[:].rearrange("p a g -> p (a g)"))
                nc.vector.tensor_copy(
                    h16[:, 2 * G:4 * G], pmax[:].rearrange("p a g -> p (a g)"))
                nc.vector.tensor_copy(
                    h16[:, 4 * G:6 * G], pmean[:].rearrange("p a g -> p (a g)"))
                nc.sync.dma_start(hout[:], h16[:])
    nc.finalize()
    return nc


# ---------------------------------------------------------------- jit runner
class _Runner:
    """Build the PJRT executable once; each call = H2D + execute + D2H."""

    def __init__(self, nc, n_cores):
        from concourse.bass2jax import (install_neuronx_cc_hook, _bass_exec_p,
                                        partition_id_tensor)
        install_neuronx_cc_hook()
        self.nc = nc
        partition_name = (nc.partition_id_tensor.name
                          if nc.partition_id_tensor else None)
        in_names, out_names, out_avals, zero_shapes = [], [], [], []
        for alloc in nc.m.functions[0].allocations:
            if not isinstance(alloc, mybir.MemoryLocationSet):
                continue
            name = alloc.memorylocations[0].name
            if alloc.kind == "ExternalInput":
                if name != partition_name:
                    in_names.append(name)
            elif alloc.kind == "ExternalOutput":
                out_names.append(name)
                shape = tuple(alloc.tensor_shape)
                dtype = mybir.dt.np(alloc.dtype)
                out_avals.append(jax.core.ShapedArray(shape, dtype))
                zero_shapes.append((shape, dtype))
        self.in_names, self.out_names = in_names, out_names
        self.zero_shapes = zero_shapes
        n_params, n_outs = len(in_names), len(out_avals)
        all_names = (list(in_names) + list(out_names)
                     + ([partition_name] if partition_name else []))

        def _body(*args):
            operands = list(args)
            if partition_name is not None:
                operands.append(partition_id_tensor())
            outs = _bass_exec_p.bind(
                *operands, out_avals=tuple(out_avals),
                in_names=tuple(all_names), out_names=tuple(out_names),
                lowering_input_output_aliases=(), sim_require_finite=True,
                sim_require_nnan=True, nc=nc)
            return tuple(outs)

        devices = jax.devices()[:n_cores]
        mesh = Mesh(np.asarray(devices), ("core",))
        in_specs = (PartitionSpec("core"),) * (n_params + n_outs)
        out_specs = (PartitionSpec("core"),) * n_outs
        self.n_cores = n_cores
        self.fn = jax.jit(
            shard_map(_body, mesh=mesh, in_specs=in_specs,
                      out_specs=out_specs, check_rep=False),
            donate_argnums=tuple(range(n_params, n_params + n_outs)),
            keep_unused=True)

    def __call__(self, in_maps):
        per_core = [[np.asarray(m[n]) for n in self.in_names] for m in in_maps]
        concat_in = [np.concatenate(
            [per_core[c][i] for c in range(self.n_cores)], axis=0)
            for i in range(len(self.in_names))]
        zeros = [np.zeros((self.n_cores * s[0], *s[1:]), d)
                 for s, d in self.zero_shapes]
        outs = self.fn(*concat_in, *zeros)
        outs = [np.asarray(o) for o in outs]
        return [{n: outs[i].reshape(self.n_cores, *self.zero_shapes[i][0])[c]
                 for i, n in enumerate(self.out_names)}
                for c in range(self.n_cores)]


# --------------------------------------------------------------------- kernel
def kernel(x, edge_index, edge_attr, batch,
           q1w, q1b, k1w, k1b, v1w, v1b, e1w, s1w, s1b, bn1w, bn1b,
           q2w, q2b, k2w, k2b, v2w, v2b, e2w, s2w, s2b, bn2w, bn2b,
           q3w, q3b, k3w, k3b, v3w, v3b, e3w, s3w, s3b, bn3w, bn3b,
           m1w, m1b, pa, m2w, m2b):
    global LAST_EXEC_NS
    x = np.asarray(x, np.float32)
    edge_index = np.asarray(edge_index)
    edge_attr = np.asarray(edge_attr, np.float32)
    batch = np.asarray(batch)
    src, dst = edge_index[0], edge_index[1]

    nblk, wstart, ea_l, dl_l, si_l = _pack_edges(src, dst, edge_attr)
    NB = wstart[-1]
    graph_parts, gcnt = _graph_segments(batch)
    SEC, TOT = _sections(NB)

    key = (tuple(nblk), tuple(tuple(p) for pl in graph_parts for p in pl))
    if key in _CACHE:
        runner = _CACHE[key]
    else:
        nc = _build_program(nblk, wstart, NB, graph_parts)
        runner = _Runner(nc, P)
        _CACHE[key] = runner

    xp = np.zeros((NT, F_IN), np.float16)
    for m in range(P):
        xp[m * NPL:m * NPL + NLOC] = x[m * NLOC:(m + 1) * NLOC]

    def f16(a):
        return np.asarray(a, np.float16)

    com = {}   # replicated sections, flat f16
    com["w1"] = f16(np.hstack([q1w, k1w, v1w, s1w])).ravel()
    com["w2"] = f16(np.hstack([q2w, k2w, v2w, s2w])).ravel()
    com["w3"] = f16(np.hstack([q3w, k3w, v3w, s3w])).ravel()
    com["b"] = f16(np.concatenate(
        [np.hstack([q1b, k1b, v1b, s1b]), np.hstack([q2b, k2b, v2b, s2b]),
         np.hstack([q3b, k3b, v3b, s3b])]))
    com["ew"] = f16(np.hstack([e1w, e2w, e3w])).ravel()
    bnp = np.zeros((128, 12), np.float16)
    for l, (bw, bb) in enumerate([(bn1w, bn1b), (bn2w, bn2b), (bn3w, bn3b)]):
        bnp[:, 2 * l:2 * l + 2] = np.asarray(bw).reshape(2, 128).T
        bnp[:, 6 + 2 * l:8 + 2 * l] = np.asarray(bb).reshape(2, 128).T
    com["bnp"] = bnp.ravel()
    rcv = np.zeros(128, np.float16)
    rcv[0:G] = (1.0 / np.maximum(gcnt, 1)).astype(np.float16)
    com["rc"] = rcv
    com["iota"] = np.arange(128, dtype=np.float16)

    in_maps = []
    for m in range(P):
        pk = np.zeros(TOT, np.float16)
        o, ne = SEC["x"]
        pk[o:o + ne] = np.ascontiguousarray(
            xp[m * NPL:(m + 1) * NPL].T).ravel()
        o, ne = SEC["ea"]
        pk[o:o + ne] = ea_l[m].ravel()
        o, ne = SEC["dl"]
        pk[o:o + ne] = dl_l[m].ravel()
        o, ne = SEC["si"]
        pk[o:o + ne] = si_l[m].ravel().view(np.float16)
        o, ne = SEC["qi"]
        ids = (m * NPL + np.arange(NPL)).astype(np.int16)
        pk[o:o + ne] = np.ascontiguousarray(
            ids.reshape(NPL // 16, 16).T).ravel().view(np.float16)
        if m == 0:
            for name, arr in com.items():
                o, ne = SEC[name]
                pk[o:o + len(arr)] = arr
        in_maps.append({"PK": pk[None, :]})

    res = runner(in_maps)
    if os.environ.get("BASS_GNN_TIME") == "1":
        t0 = time.perf_counter_ns()
        res = runner(in_maps)
        LAST_EXEC_NS = time.perf_counter_ns() - t0

    ho = np.asarray(res[0]["hout"], np.float32)   # [128, 384]
    x_add = np.empty((G, HC), np.float32)
    x_max = np.empty((G, HC), np.float32)
    x_mean = np.empty((G, HC), np.float32)
    for i, arr in enumerate([x_add, x_max, x_mean]):
        blk = ho[:, i * 2 * G:(i + 1) * 2 * G].reshape(128, 2, G)
        arr[:, 0:128] = blk[:, 0, :].T
        arr[:, 128:256] = blk[:, 1, :].T

    h = np.concatenate([x_add, x_max, x_mean], axis=1).astype(np.float32)
    h = h @ np.asarray(m1w, np.float32) + np.asarray(m1b, np.float32)
    h = np.where(h >= 0, h, np.float32(pa) * h)
    lg = h @ np.asarray(m2w, np.float32) + np.asarray(m2b, np.float32)
    mx = lg.max(axis=1, keepdims=True)
    sh = lg - mx
    return (sh - np.log(np.exp(sh).sum(axis=1, keepdims=True))).astype(np.float32)


# revision 19
# speedup vs baseline: 9.7542x; 1.3123x over previous
"""Full 3-layer TransformerConv GNN on 8 Trainium2 cores.

Sharding: edges sorted by dst and partitioned into 8 contiguous dst-node
ranges (2500 nodes/core, padded to 2560).  Node projections (q/k/v/skip)
are computed replicated on every core into global DRAM tables; each core
runs segment-softmax message aggregation only for its 20 local 128-node
dst windows via one-hot scatter matmuls (PSUM-accumulated per window).
Pre-BN layer outputs are AllGathered (feat-major) between layers; BN
statistics are computed replicated from the gathered tensor.  Per-graph
sum/max/mean pooling happens on device; only the [64,768] pooled tensor
returns to the host, which applies the tiny MLP head.

All host->device payload travels in ONE packed fp16 tensor per core
(int16 index sections bitcast) to minimize axon-tunnel transfer time,
which dominates the dispatch wall clock.  Device compute stays fp32.

Self-contained: shapes hardcoded, sharding derived from the inputs.
"""
import math
import os
import time
import numpy as np

import jax
from jax.sharding import Mesh, PartitionSpec
from jax.experimental.shard_map import shard_map

from concourse import bacc, bass, tile, mybir, library_config

P = 8
N, E, F_IN, ED, G = 20000, 640000, 128, 4, 64
HC = 256
NLOC = N // P          # 2500
NWIN = 20              # 128-node dst windows per core
NPL = NWIN * 128       # 2560 padded local nodes
NT = P * NPL           # 20480 padded global nodes
EPS = 1e-5
F32 = mybir.dt.float32
F16 = mybir.dt.float16
I16 = mybir.dt.int16

LAST_EXEC_NS = None

_CACHE = {}


# ----------------------------------------------------------------- host pack
def _pack_edges(src, dst, edge_attr):
    order = np.argsort(dst, kind="stable")
    so = src[order].astype(np.int64)
    do = dst[order].astype(np.int64)
    eao = edge_attr[order].astype(np.float32)

    core = do // NLOC
    wloc = (do - core * NLOC) >> 7          # local window [0, NWIN)
    cw = np.zeros((P, NWIN), np.int64)
    np.add.at(cw, (core, wloc), 1)
    nblk = (-(-cw // 128)).max(axis=0)      # common blocks per window
    wstart = np.zeros(NWIN + 1, np.int64)
    wstart[1:] = np.cumsum(nblk)
    NB = int(wstart[-1])
    EP = NB * 128

    key = core * NWIN + wloc
    kcounts = np.bincount(key, minlength=P * NWIN)
    kstarts = np.zeros(P * NWIN, np.int64)
    kstarts[1:] = np.cumsum(kcounts)[:-1]
    ko = np.argsort(key, kind="stable")
    pos = np.empty(len(so), np.int64)
    pos[ko] = np.arange(len(so)) - kstarts[key[ko]]
    slot = wstart[wloc] * 128 + pos
    fi = core * EP + slot

    flat_ea = np.zeros((P * EP, ED), np.float16)
    flat_dl = np.full(P * EP, -1.0, np.float16)
    flat_si = np.zeros(P * EP, np.int64)
    flat_ea[fi] = eao
    flat_dl[fi] = (do - core * NLOC - wloc * 128).astype(np.float16)
    sc = so // NLOC
    flat_si[fi] = sc * NPL + (so - sc * NLOC)

    ea_l, dl_l, si_l = [], [], []
    for m in range(P):
        ea_l.append(np.ascontiguousarray(flat_ea[m * EP:(m + 1) * EP].T))
        dl_l.append(np.ascontiguousarray(
            flat_dl[m * EP:(m + 1) * EP].reshape(NB, 128).T))
        s16 = flat_si[m * EP:(m + 1) * EP].astype(np.int16)
        si_l.append(np.ascontiguousarray(s16.reshape(EP // 16, 16).T))
    return [int(v) for v in nblk], [int(v) for v in wstart], ea_l, dl_l, si_l


def _graph_segments(batch):
    gcnt = np.bincount(batch, minlength=G)
    assert (gcnt > 0).all(), "empty graph segment"
    gstart = np.zeros(G + 1, np.int64)
    gstart[1:] = np.cumsum(gcnt)
    parts = []  # per graph: list of (rank, lo, ln)
    for g in range(G):
        s, e = int(gstart[g]), int(gstart[g + 1])
        pl = []
        for r in range(s // NLOC, (e - 1) // NLOC + 1):
            lo = max(s, r * NLOC) - r * NLOC
            hi = min(e, (r + 1) * NLOC) - r * NLOC
            pl.append((r, lo, hi - lo))
        assert 1 <= len(pl) <= 2
        parts.append(pl)
    return parts, gcnt


def _wall_layout():
    """Replicated payload (weights etc), AllGathered on device from
    per-core 1/8 shards.  name -> (offset, nelem); 128-elem aligned."""
    sizes = [
        ("w1", F_IN * 4 * HC),
        ("w2", HC * 4 * HC),
        ("w3", HC * 4 * HC),
        ("b", 3 * 4 * HC),
        ("ew", ED * 3 * HC),
        ("bnp", 128 * 12),
        ("rc", 128),           # per-graph 1/count row (G used)
        ("iota", 128),         # row 0..127
    ]
    off, out = 0, {}
    for name, ne in sizes:
        out[name] = (off, ne)
        off += (ne + 127) // 128 * 128
    wall = (off + 8 * 128 - 1) // (8 * 128) * (8 * 128)
    return out, wall


def _sections(NB):
    """Packed-tensor layout: name -> (offset, nelem), 128-elem aligned."""
    EP = NB * 128
    _, WALL = _wall_layout()
    sizes = [
        ("x", F_IN * NPL),
        ("ea", ED * EP),
        ("dl", 128 * NB),
        ("si", EP),            # [16, EP/16] int16
        ("qi", NPL),           # [16, NPL/16] int16
        ("wsh", WALL // 8),    # this core's shard of the replicated wall
    ]
    off, out = 0, {}
    for name, ne in sizes:
        out[name] = (off, ne)
        off += (ne + 127) // 128 * 128
    return out, off


# -------------------------------------------------------------- bass program
def _build_program(nblk, wstart, NB, graph_parts):
    STOP = os.environ.get("BASS_GNN_STOP", "full")

    def _stopped(tag):
        order = ["l0proj", "l0edge", "l0post", "l1proj", "l1edge", "l1post",
                 "l2proj", "l2edge", "l2post", "pool", "full"]
        return order.index(STOP) < order.index(tag)

    EP = NB * 128
    SEC, TOT = _sections(NB)
    nc = bacc.Bacc("TRN2", debug=False, num_devices=P)

    PK = nc.dram_tensor("PK", [1, TOT], F16, kind="ExternalInput")
    hout = nc.dram_tensor("hout", [128, 3 * 2 * G], F16, kind="ExternalOutput")
    PKt = PK[:].tensor

    def sec2d(name, p, f, sub_off=0):
        if name in SECW:
            off, ne = SECW[name]
            assert p * f + sub_off <= ne
            return bass.AP(tensor=wbuf[:].tensor, offset=off + sub_off,
                           ap=[[f, p], [1, f]])
        off, ne = SEC[name]
        assert p * f + sub_off <= ne
        return bass.AP(tensor=PKt, offset=off + sub_off, ap=[[f, p], [1, f]])

    def wsec_off(name):
        return SECW[name][0]

    SECW, WALL = _wall_layout()
    wpart = nc.dram_tensor("wpart", [1, WALL // 8], F16, kind="Internal")
    wbuf = nc.dram_tensor("wbuf", [1, WALL], F16, kind="Internal",
                          addr_space="Shared")
    cc1 = nc.dram_tensor("cc1", [F_IN, NPL], F16, kind="Internal")
    xg1 = nc.dram_tensor("xg1", [P, F_IN, NPL], F16, kind="Internal",
                         addr_space="Shared")
    xg = [nc.dram_tensor(f"xg{l+2}", [P, HC, NPL], F32, kind="Internal",
                         addr_space="Shared") for l in range(3)]
    qtab = [nc.dram_tensor(f"qtab{l}", [NT, HC], F32, kind="Internal")
            for l in range(3)]
    ktab = [nc.dram_tensor(f"ktab{l}", [NT, HC], F32, kind="Internal")
            for l in range(3)]
    vtab = [nc.dram_tensor(f"vtab{l}", [NT, HC], F32, kind="Internal")
            for l in range(3)]
    stab = [nc.dram_tensor(f"stab{l}", [NT, HC], F32, kind="Internal")
            for l in range(3)]
    aggT = [nc.dram_tensor(f"aggT{l}", [HC, NPL], F32, kind="Internal")
            for l in range(3)]

    AG = mybir.AluOpType
    AF = mybir.ActivationFunctionType
    groups = [list(range(P))]
    holds = {}

    with tile.TileContext(nc) as tc:
        nc.gpsimd.load_library(library_config.mlp)
        nc.sync.dma_start(
            wpart[:],
            bass.AP(tensor=PKt, offset=SEC["wsh"][0],
                    ap=[[1, 1], [1, WALL // 8]]))
        nc.gpsimd.collective_compute(
            "AllGather", mybir.AluOpType.bypass, replica_groups=groups,
            ins=[wpart[:]], outs=[wbuf[:]])
        with tc.tile_pool(name="const", bufs=1) as cp:
            # iota row -> broadcast tiles, identity, tail mask
            o_iota = wsec_off("iota")
            io16 = cp.tile([128, 128], F16, name="io16")
            nc.gpsimd.dma_start(io16[:], bass.AP(
                tensor=wbuf[:].tensor, offset=o_iota, ap=[[0, 128], [1, 128]]))
            io32 = cp.tile([128, 128], F32, name="io32")
            nc.vector.tensor_copy(io32[:], io16[:])
            ioc16 = cp.tile([128, 1], F16, name="ioc16")
            nc.gpsimd.dma_start(ioc16[:], bass.AP(
                tensor=wbuf[:].tensor, offset=o_iota, ap=[[1, 128], [1, 1]]))
            ioc32 = cp.tile([128, 1], F32, name="ioc32")
            nc.vector.tensor_copy(ioc32[:], ioc16[:])
            s_eye = cp.tile([128, 128], F32, name="s_eye")
            nc.vector.tensor_scalar(s_eye[:], io32[:], ioc32[:], None,
                                    AG.is_equal)
            s_MASK = cp.tile([128, 1], F32, name="s_MASK")
            nc.vector.tensor_scalar(
                s_MASK[:], ioc32[:], float(NLOC - (NWIN - 1) * 128), None,
                AG.is_lt)

            # weights -> f32 SBUF
            s_W = []
            for l, (wn, K) in enumerate([("w1", F_IN), ("w2", HC), ("w3", HC)]):
                tiles = []
                for kh in range(K // 128):
                    t16 = cp.tile([128, 4 * HC], F16, name=f"w16_{l}_{kh}",
                                  tag="w16stage")
                    nc.sync.dma_start(
                        t16[:], sec2d(wn, 128, 4 * HC,
                                      sub_off=kh * 128 * 4 * HC))
                    t = cp.tile([128, 4 * HC], F32, name=f"s_W{l}_{kh}")
                    nc.vector.tensor_copy(t[:], t16[:])
                    tiles.append(t)
                s_W.append(tiles)
            s_B = []
            o_b = wsec_off("b")
            for l in range(3):
                t16 = cp.tile([128, 4 * HC], F16, name=f"b16_{l}",
                              tag="w16stage")
                nc.gpsimd.dma_start(t16[:], bass.AP(
                    tensor=wbuf[:].tensor, offset=o_b + l * 4 * HC,
                    ap=[[0, 128], [1, 4 * HC]]))
                t = cp.tile([128, 4 * HC], F32, name=f"s_B{l}")
                nc.vector.tensor_copy(t[:], t16[:])
                s_B.append(t)
            s_EW = cp.tile([ED, 3 * HC], F16, name="s_EW")
            nc.sync.dma_start(s_EW[:], sec2d("ew", ED, 3 * HC))
            bnp16 = cp.tile([128, 12], F16, name="bnp16")
            nc.sync.dma_start(bnp16[:], sec2d("bnp", 128, 12))
            s_BNP = cp.tile([128, 12], F32, name="s_BNP")
            nc.vector.tensor_copy(s_BNP[:], bnp16[:])
            rc16 = cp.tile([128, G], F16, name="rc16")
            nc.gpsimd.dma_start(rc16[:], bass.AP(
                tensor=wbuf[:].tensor, offset=wsec_off("rc"),
                ap=[[0, 128], [1, G]]))
            s_RC = cp.tile([128, G], F32, name="s_RC")
            nc.vector.tensor_copy(s_RC[:], rc16[:])

            dl16 = cp.tile([128, NB], F16, name="dl16")
            nc.sync.dma_start(dl16[:], sec2d("dl", 128, NB))
            s_dstl = cp.tile([128, NB], F32, name="s_dstl")
            nc.vector.tensor_copy(s_dstl[:], dl16[:])
            # int16 index sections: [16, C] on wire -> replicate to 128 rows
            C = EP // 16
            s_sidx = cp.tile([128, C], I16, name="s_sidx")
            nc.gpsimd.dma_start(s_sidx[:], bass.AP(
                tensor=PKt, offset=SEC["si"][0],
                ap=[[0, 8], [C, 16], [1, C]]).bitcast(I16))
            CQ = NPL // 16
            s_qidx = cp.tile([128, CQ], I16, name="s_qidx")
            nc.gpsimd.dma_start(s_qidx[:], bass.AP(
                tensor=PKt, offset=SEC["qi"][0],
                ap=[[0, 8], [CQ, 16], [1, CQ]]).bitcast(I16))

            # ------------------------------------------- x AllGather (layer 1)
            nc.sync.dma_start(
                cc1[:].rearrange("a b -> (a b)").unsqueeze(0),
                bass.AP(tensor=PKt, offset=SEC["x"][0],
                        ap=[[1, 1], [1, F_IN * NPL]]))
            nc.gpsimd.collective_compute(
                "AllGather", AG.bypass, replica_groups=groups,
                ins=[cc1[:]], outs=[xg1[:]])

            sbn = [None, None, None]
            tbn = [None, None, None]
            sbn3 = tbn3 = None

            # ======================================================= layers
            for l in range(3):
                if _stopped(f"l{l}proj"):
                    break
                H = 4 if l == 0 else 1
                C_h = 64 if l == 0 else HC
                inv_sqrt_c = 1.0 / math.sqrt(C_h)
                K = F_IN if l == 0 else HC
                src_g = xg1 if l == 0 else xg[l - 1]

                # ---- projections: q/k/v/s tables for all NT nodes
                with (tc.tile_pool(name=f"pj{l}", bufs=3) as wp,
                      tc.tile_pool(name=f"pjp{l}", bufs=2, space="PSUM") as pp):
                    for r in range(P):
                        for c in range(NPL // 512):
                            xts = []
                            for kh in range(K // 128):
                                if l == 0:
                                    x16 = wp.tile([128, 512], F16,
                                                  tag=f"x16_{kh}")
                                    nc.sync.dma_start(
                                        x16[:],
                                        src_g[r, kh * 128:(kh + 1) * 128,
                                              c * 512:(c + 1) * 512])
                                    xb = wp.tile([128, 512], F32,
                                                 tag=f"xb{kh}")
                                    nc.vector.tensor_copy(xb[:], x16[:])
                                    xts.append(xb)
                                else:
                                    xt = wp.tile([128, 512], F32,
                                                 tag=f"xt{kh}")
                                    nc.sync.dma_start(
                                        xt[:],
                                        src_g[r, kh * 128:(kh + 1) * 128,
                                              c * 512:(c + 1) * 512])
                                    xb = wp.tile([128, 512], F32,
                                                 tag=f"xb{kh}")
                                    nc.scalar.activation(
                                        xb[:], xt[:], AF.Identity,
                                        bias=tbn[l][:, kh:kh + 1],
                                        scale=sbn[l][:, kh:kh + 1])
                                    xts.append(xb)
                            for sub in range(4):
                                row0 = r * NPL + c * 512 + sub * 128
                                for half in range(2):
                                    ppt = pp.tile(
                                        [128, 512], F32, tag=f"pp{half}",
                                        name=f"pp{l}_{r}_{c}_{sub}_{half}")
                                    nkh = K // 128
                                    for kh in range(nkh):
                                        nc.tensor.matmul(
                                            ppt[:],
                                            xts[kh][:, sub * 128:(sub + 1) * 128],
                                            s_W[l][kh][:, half * 512:(half + 1) * 512],
                                            start=(kh == 0),
                                            stop=(kh == nkh - 1))
                                    ob = wp.tile([128, 512], F32, tag="ob")
                                    nc.vector.tensor_tensor(
                                        ob[:], ppt[:],
                                        s_B[l][:, half * 512:(half + 1) * 512],
                                        AG.add)
                                    if half == 0:
                                        nc.sync.dma_start(
                                            qtab[l][row0:row0 + 128, :],
                                            ob[:, 0:HC])
                                        nc.sync.dma_start(
                                            ktab[l][row0:row0 + 128, :],
                                            ob[:, HC:2 * HC])
                                    else:
                                        nc.sync.dma_start(
                                            vtab[l][row0:row0 + 128, :],
                                            ob[:, 0:HC])
                                        nc.sync.dma_start(
                                            stab[l][row0:row0 + 128, :],
                                            ob[:, HC:2 * HC])

                # ---- edge phase: per-window segment softmax + aggregation
                if _stopped(f"l{l}edge"):
                    break
                ewsl = s_EW[:, l * HC:(l + 1) * HC]
                with (tc.tile_pool(name=f"ed{l}", bufs=3) as wp,
                      tc.tile_pool(name=f"edp{l}", bufs=2, space="PSUM") as pe):
                    for w in range(NWIN):
                        qwg = wp.tile([128, 1, HC], F32, tag="qw")
                        nc.gpsimd.dma_gather(
                            qwg[:], qtab[l][:], s_qidx[:, w * 8:(w + 1) * 8],
                            128, 128, HC)
                        swg = wp.tile([128, 1, HC], F32, tag="sw")
                        nc.gpsimd.dma_gather(
                            swg[:], stab[l][:], s_qidx[:, w * 8:(w + 1) * 8],
                            128, 128, HC)
                        qw = qwg[:].rearrange("p a f -> p (a f)")
                        sw = swg[:].rearrange("p a f -> p (a f)")
                        nb = nblk[w]
                        pagg = (pe.tile([128, HC + H], F32, tag="agg",
                                        name=f"pagg{l}_{w}")
                                if nb > 0 else None)
                        for b in range(nb):
                            blk = wstart[w] + b
                            S = wp.tile([128, 128], F32, tag="S")
                            nc.vector.tensor_scalar(
                                S[:], io32[:], s_dstl[:, blk:blk + 1], None,
                                AG.is_equal)
                            pS = pe.tile([128, 128], F32, tag="pS")
                            nc.tensor.transpose(pS[:], S[:], s_eye[:])
                            ST = wp.tile([128, 128], F32, tag="ST")
                            nc.scalar.copy(ST[:], pS[:])
                            kg = wp.tile([128, 1, HC], F32, tag="kg")
                            nc.gpsimd.dma_gather(
                                kg[:], ktab[l][:],
                                s_sidx[:, blk * 8:(blk + 1) * 8], 128, 128, HC)
                            vg = wp.tile([128, 1, HC], F32, tag="vg")
                            nc.gpsimd.dma_gather(
                                vg[:], vtab[l][:],
                                s_sidx[:, blk * 8:(blk + 1) * 8], 128, 128, HC)
                            kg2 = kg[:].rearrange("p a f -> p (a f)")
                            vg2 = vg[:].rearrange("p a f -> p (a f)")
                            eat = wp.tile([ED, 128], F16, tag="eat")
                            nc.sync.dma_start(
                                eat[:], sec2d("ea", ED, EP)[
                                    :, blk * 128:(blk + 1) * 128])
                            pE = pe.tile([128, HC], F32, tag="pE")
                            nc.tensor.matmul(pE[:], eat[:], ewsl,
                                             start=True, stop=True)
                            pQ = pe.tile([128, HC], F32, tag="pQ")
                            nc.tensor.matmul(pQ[:], ST[:], qw,
                                             start=True, stop=True)
                            kj = wp.tile([128, HC], F32, tag="kj")
                            nc.vector.tensor_tensor(kj[:], kg2, pE[:], AG.add)
                            prod = wp.tile([128, HC], F32, tag="prod")
                            nc.vector.tensor_tensor(prod[:], pQ[:], kj[:],
                                                    AG.mult)
                            al = wp.tile([128, H], F32, tag="al")
                            nc.vector.tensor_reduce(
                                al[:],
                                prod[:].rearrange("p (h c) -> p h c", h=H),
                                mybir.AxisListType.X, AG.add)
                            ex = wp.tile([128, H], F32, tag="ex")
                            nc.scalar.activation(ex[:], al[:], AF.Exp,
                                                 scale=inv_sqrt_c)
                            vj = wp.tile([128, HC], F32, tag="vj")
                            nc.vector.tensor_tensor(vj[:], vg2, pE[:], AG.add)
                            mv = wp.tile([128, HC + H], F32, tag="mv")
                            if H == 1:
                                nc.vector.tensor_scalar_mul(mv[:, 0:HC], vj[:],
                                                            ex[:, 0:1])
                            else:
                                nc.vector.tensor_tensor(
                                    mv[:, 0:HC].rearrange(
                                        "p (h c) -> p h c", h=H),
                                    vj[:].rearrange("p (h c) -> p h c", h=H),
                                    ex[:].unsqueeze(2).broadcast_to(
                                        [128, H, C_h]),
                                    AG.mult)
                            nc.vector.tensor_copy(mv[:, HC:HC + H], ex[:])
                            nc.tensor.matmul(pagg[:, 0:HC + H], S[:], mv[:],
                                             start=(b == 0),
                                             stop=(b == nb - 1))
                        # window evacuation
                        ob2 = wp.tile([128, HC], F32, tag="ob2")
                        if nb == 0:
                            nc.vector.tensor_copy(ob2[:], sw)
                        else:
                            den = wp.tile([128, H], F32, tag="den")
                            nc.vector.tensor_scalar_add(
                                den[:], pagg[:, HC:HC + H], 1e-16)
                            rc = wp.tile([128, H], F32, tag="rc")
                            nc.vector.reciprocal(rc[:], den[:])
                            ob = wp.tile([128, HC], F32, tag="ob")
                            if H == 1:
                                nc.vector.tensor_scalar_mul(
                                    ob[:], pagg[:, 0:HC], rc[:, 0:1])
                            else:
                                nc.vector.tensor_tensor(
                                    ob[:].rearrange("p (h c) -> p h c", h=H),
                                    pagg[:, 0:HC].rearrange(
                                        "p (h c) -> p h c", h=H),
                                    rc[:].unsqueeze(2).broadcast_to(
                                        [128, H, C_h]),
                                    AG.mult)
                            nc.vector.tensor_tensor(ob2[:], ob[:], sw, AG.add)
                        if w == NWIN - 1:
                            obm = wp.tile([128, HC], F32, tag="obm")
                            nc.vector.tensor_scalar_mul(obm[:], ob2[:],
                                                        s_MASK[:, 0:1])
                        else:
                            obm = ob2
                        for fh in range(2):
                            pt = pe.tile([128, 128], F32, tag="pS",
                                         name=f"pt{l}_{w}_{fh}")
                            nc.tensor.transpose(
                                pt[:], obm[:, fh * 128:(fh + 1) * 128],
                                s_eye[:])
                            tb = wp.tile([128, 128], F32, tag="tb")
                            nc.vector.tensor_copy(tb[:], pt[:])
                            nc.sync.dma_start(
                                aggT[l][fh * 128:(fh + 1) * 128,
                                        w * 128:(w + 1) * 128], tb[:])

                # ---- AllGather pre-BN output, then global BN stats
                if _stopped(f"l{l}post"):
                    break
                nc.gpsimd.collective_compute(
                    "AllGather", AG.bypass, replica_groups=groups,
                    ins=[aggT[l][:]], outs=[xg[l][:]])

                with tc.tile_pool(name=f"st{l}", bufs=2) as wp:
                    sums = wp.tile([128, 2, P], F32, tag="sums")
                    sqs = wp.tile([128, 2, P], F32, tag="sqs")
                    for r in range(P):
                        for fh in range(2):
                            ch = wp.tile([128, NPL], F32, tag="ch")
                            nc.sync.dma_start(
                                ch[:], xg[l][r, fh * 128:(fh + 1) * 128, :])
                            nc.vector.tensor_reduce(
                                sums[:, fh:fh + 1, r:r + 1].rearrange(
                                    "p a b -> p (a b)"),
                                ch[:], mybir.AxisListType.X, AG.add)
                            scr = wp.tile([128, NPL], F32, tag="scr")
                            nc.scalar.activation(
                                scr[:], ch[:], AF.Square,
                                accum_out=sqs[:, fh:fh + 1, r:r + 1].rearrange(
                                    "p a b -> p (a b)"))
                    musum = wp.tile([128, 2], F32, tag="musum")
                    nc.vector.tensor_reduce(musum[:], sums[:],
                                            mybir.AxisListType.X, AG.add)
                    mu = cp.tile([128, 2], F32, name=f"mu{l}")
                    nc.vector.tensor_scalar_mul(mu[:], musum[:], 1.0 / N)
                    sqsum = wp.tile([128, 2], F32, tag="sqsum")
                    nc.vector.tensor_reduce(sqsum[:], sqs[:],
                                            mybir.AxisListType.X, AG.add)
                    ex2 = wp.tile([128, 2], F32, tag="ex2")
                    nc.vector.tensor_scalar_mul(ex2[:], sqsum[:], 1.0 / N)
                    m2 = wp.tile([128, 2], F32, tag="m2")
                    nc.vector.tensor_tensor(m2[:], mu[:], mu[:], AG.mult)
                    var = wp.tile([128, 2], F32, tag="var")
                    nc.vector.tensor_tensor(var[:], ex2[:], m2[:], AG.subtract)
                    vpe = wp.tile([128, 2], F32, tag="vpe")
                    nc.vector.tensor_scalar_add(vpe[:], var[:], EPS)
                    sd = wp.tile([128, 2], F32, tag="sd")
                    nc.scalar.activation(sd[:], vpe[:], AF.Sqrt)
                    rstd = wp.tile([128, 2], F32, tag="rstd")
                    nc.vector.reciprocal(rstd[:], sd[:])
                    sb_t = cp.tile([128, 2], F32, name=f"sbn{l}")
                    nc.vector.tensor_tensor(sb_t[:], rstd[:],
                                            s_BNP[:, 2 * l:2 * l + 2], AG.mult)
                    tmp = wp.tile([128, 2], F32, tag="tmp")
                    nc.vector.tensor_tensor(tmp[:], mu[:], sb_t[:], AG.mult)
                    tb_t = cp.tile([128, 2], F32, name=f"tbn{l}")
                    nc.vector.tensor_tensor(
                        tb_t[:], s_BNP[:, 6 + 2 * l:8 + 2 * l], tmp[:],
                        AG.subtract)
                    if l < 2:
                        sbn[l + 1] = sb_t
                        tbn[l + 1] = tb_t
                    else:
                        sbn3, tbn3 = sb_t, tb_t

            # =============================================== pooling (layer 3)
            with tc.tile_pool(name="pool", bufs=2) as wp:
              if not _stopped("pool"):
                padd = wp.tile([128, 2, G], F32, tag="padd")
                pmax = wp.tile([128, 2, G], F32, tag="pmax")
                for fh in range(2):
                    for r in range(P):
                        ch = wp.tile([128, NLOC], F32, tag="pch")
                        nc.sync.dma_start(
                            ch[:], xg[2][r, fh * 128:(fh + 1) * 128, 0:NLOC])
                        bnc = wp.tile([128, NLOC], F32, tag="pbn")
                        nc.scalar.activation(bnc[:], ch[:], AF.Identity,
                                             bias=tbn3[:, fh:fh + 1],
                                             scale=sbn3[:, fh:fh + 1])
                        for g, pl in enumerate(graph_parts):
                            for (pr, lo, ln) in pl:
                                if pr != r:
                                    continue
                                seg = bnc[:, lo:lo + ln]
                                if len(pl) == 1:
                                    nc.vector.tensor_reduce(
                                        padd[:, fh:fh + 1, g:g + 1].rearrange(
                                            "p a b -> p (a b)"),
                                        seg, mybir.AxisListType.X, AG.add)
                                    nc.vector.tensor_reduce(
                                        pmax[:, fh:fh + 1, g:g + 1].rearrange(
                                            "p a b -> p (a b)"),
                                        seg, mybir.AxisListType.X, AG.max)
                                else:
                                    first = (pr, lo, ln) == pl[0]
                                    sfx = "a" if first else "b"
                                    ta = wp.tile([128, 1], F32,
                                                 tag=f"t{sfx}_add",
                                                 name=f"t{sfx}a_{fh}_{g}")
                                    nc.vector.tensor_reduce(
                                        ta[:], seg, mybir.AxisListType.X,
                                        AG.add)
                                    tm = wp.tile([128, 1], F32,
                                                 tag=f"t{sfx}_max",
                                                 name=f"t{sfx}m_{fh}_{g}")
                                    nc.vector.tensor_reduce(
                                        tm[:], seg, mybir.AxisListType.X,
                                        AG.max)
                                    if first:
                                        holds[(fh, g)] = (ta, tm)
                                    else:
                                        ha, hm = holds.pop((fh, g))
                                        nc.vector.tensor_tensor(
                                            padd[:, fh:fh + 1,
                                                 g:g + 1].rearrange(
                                                "p a b -> p (a b)"),
                                            ha[:], ta[:], AG.add)
                                        nc.vector.tensor_tensor(
                                            pmax[:, fh:fh + 1,
                                                 g:g + 1].rearrange(
                                                "p a b -> p (a b)"),
                                            hm[:], tm[:], AG.max)
                pmean = wp.tile([128, 2, G], F32, tag="pmean")
                for fh in range(2):
                    nc.vector.tensor_tensor(
                        pmean[:, fh, :], padd[:, fh, :], s_RC[:], AG.mult)
                h16 = wp.tile([128, 3 * 2 * G], F16, tag="h16")
                nc.vector.tensor_copy(
                    h16[:, 0:2 * G], padd[:].rearrange("p a g -> p (a g)"))
                nc.vector.tensor_copy(
                    h16[:, 2 * G:4 * G], pmax[:].rearrange("p a g -> p (a g)"))
                nc.vector.tensor_copy(
                    h16[:, 4 * G:6 * G], pmean[:].rearrange("p a g -> p (a g)"))
                nc.sync.dma_start(hout[:], h16[:])
    nc.finalize()
    return nc


# ---------------------------------------------------------------- jit runner
class _Runner:
    """Build the PJRT executable once; each call = H2D + execute + D2H."""

    def __init__(self, nc, n_cores):
        from concourse.bass2jax import (install_neuronx_cc_hook, _bass_exec_p,
                                        partition_id_tensor)
        install_neuronx_cc_hook()
        self.nc = nc
        partition_name = (nc.partition_id_tensor.name
                          if nc.partition_id_tensor else None)
        in_names, out_names, out_avals, zero_shapes = [], [], [], []
        for alloc in nc.m.functions[0].allocations:
            if not isinstance(alloc, mybir.MemoryLocationSet):
                continue
            name = alloc.memorylocations[0].name
            if alloc.kind == "ExternalInput":
                if name != partition_name:
                    in_names.append(name)
            elif alloc.kind == "ExternalOutput":
                out_names.append(name)
                shape = tuple(alloc.tensor_shape)
                dtype = mybir.dt.np(alloc.dtype)
                out_avals.append(jax.core.ShapedArray(shape, dtype))
                zero_shapes.append((shape, dtype))
        self.in_names, self.out_names = in_names, out_names
        self.zero_shapes = zero_shapes
        n_params, n_outs = len(in_names), len(out_avals)
        all_names = (list(in_names) + list(out_names)
                     + ([partition_name] if partition_name else []))

        def _body(*args):
            operands = list(args)
            if partition_name is not None:
                operands.append(partition_id_tensor())
            outs = _bass_exec_p.bind(
                *operands, out_avals=tuple(out_avals),
                in_names=tuple(all_names), out_names=tuple(out_names),
                lowering_input_output_aliases=(), sim_require_finite=True,
                sim_require_nnan=True, nc=nc)
            return tuple(outs)

        devices = jax.devices()[:n_cores]
        mesh = Mesh(np.asarray(devices), ("core",))
        in_specs = (PartitionSpec("core"),) * (n_params + n_outs)
        out_specs = (PartitionSpec("core"),) * n_outs
        self.n_cores = n_cores
        self.fn = jax.jit(
            shard_map(_body, mesh=mesh, in_specs=in_specs,
                      out_specs=out_specs, check_rep=False),
            donate_argnums=tuple(range(n_params, n_params + n_outs)),
            keep_unused=True)

    def __call__(self, in_maps):
        per_core = [[np.asarray(m[n]) for n in self.in_names] for m in in_maps]
        concat_in = [np.concatenate(
            [per_core[c][i] for c in range(self.n_cores)], axis=0)
            for i in range(len(self.in_names))]
        zeros = [np.zeros((self.n_cores * s[0], *s[1:]), d)
                 for s, d in self.zero_shapes]
        outs = self.fn(*concat_in, *zeros)
        outs = [np.asarray(o) for o in outs]
        return [{n: outs[i].reshape(self.n_cores, *self.zero_shapes[i][0])[c]
                 for i, n in enumerate(self.out_names)}
                for c in range(self.n_cores)]


# --------------------------------------------------------------------- kernel
def kernel(x, edge_index, edge_attr, batch,
           q1w, q1b, k1w, k1b, v1w, v1b, e1w, s1w, s1b, bn1w, bn1b,
           q2w, q2b, k2w, k2b, v2w, v2b, e2w, s2w, s2b, bn2w, bn2b,
           q3w, q3b, k3w, k3b, v3w, v3b, e3w, s3w, s3b, bn3w, bn3b,
           m1w, m1b, pa, m2w, m2b):
    global LAST_EXEC_NS
    x = np.asarray(x, np.float32)
    edge_index = np.asarray(edge_index)
    edge_attr = np.asarray(edge_attr, np.float32)
    batch = np.asarray(batch)
    src, dst = edge_index[0], edge_index[1]

    nblk, wstart, ea_l, dl_l, si_l = _pack_edges(src, dst, edge_attr)
    NB = wstart[-1]
    graph_parts, gcnt = _graph_segments(batch)
    SEC, TOT = _sections(NB)

    key = (tuple(nblk), tuple(tuple(p) for pl in graph_parts for p in pl))
    if key in _CACHE:
        runner = _CACHE[key]
    else:
        nc = _build_program(nblk, wstart, NB, graph_parts)
        runner = _Runner(nc, P)
        _CACHE[key] = runner

    xp = np.zeros((NT, F_IN), np.float16)
    for m in range(P):
        xp[m * NPL:m * NPL + NLOC] = x[m * NLOC:(m + 1) * NLOC]

    def f16(a):
        return np.asarray(a, np.float16)

    com = {}   # replicated sections, flat f16
    com["w1"] = f16(np.hstack([q1w, k1w, v1w, s1w])).ravel()
    com["w2"] = f16(np.hstack([q2w, k2w, v2w, s2w])).ravel()
    com["w3"] = f16(np.hstack([q3w, k3w, v3w, s3w])).ravel()
    com["b"] = f16(np.concatenate(
        [np.hstack([q1b, k1b, v1b, s1b]), np.hstack([q2b, k2b, v2b, s2b]),
         np.hstack([q3b, k3b, v3b, s3b])]))
    com["ew"] = f16(np.hstack([e1w, e2w, e3w])).ravel()
    bnp = np.zeros((128, 12), np.float16)
    for l, (bw, bb) in enumerate([(bn1w, bn1b), (bn2w, bn2b), (bn3w, bn3b)]):
        bnp[:, 2 * l:2 * l + 2] = np.asarray(bw).reshape(2, 128).T
        bnp[:, 6 + 2 * l:8 + 2 * l] = np.asarray(bb).reshape(2, 128).T
    com["bnp"] = bnp.ravel()
    rcv = np.zeros(128, np.float16)
    rcv[0:G] = (1.0 / np.maximum(gcnt, 1)).astype(np.float16)
    com["rc"] = rcv
    com["iota"] = np.arange(128, dtype=np.float16)

    SECW, WALL = _wall_layout()
    wall = np.zeros(WALL, np.float16)
    for name, arr in com.items():
        o, ne = SECW[name]
        wall[o:o + len(arr)] = arr
    WSH = WALL // 8

    in_maps = []
    for m in range(P):
        pk = np.zeros(TOT, np.float16)
        o, ne = SEC["x"]
        pk[o:o + ne] = np.ascontiguousarray(
            xp[m * NPL:(m + 1) * NPL].T).ravel()
        o, ne = SEC["ea"]
        pk[o:o + ne] = ea_l[m].ravel()
        o, ne = SEC["dl"]
        pk[o:o + ne] = dl_l[m].ravel()
        o, ne = SEC["si"]
        pk[o:o + ne] = si_l[m].ravel().view(np.float16)
        o, ne = SEC["qi"]
        ids = (m * NPL + np.arange(NPL)).astype(np.int16)
        pk[o:o + ne] = np.ascontiguousarray(
            ids.reshape(NPL // 16, 16).T).ravel().view(np.float16)
        o, ne = SEC["wsh"]
        pk[o:o + WSH] = wall[m * WSH:(m + 1) * WSH]
        in_maps.append({"PK": pk[None, :]})

    res = runner(in_maps)
    if os.environ.get("BASS_GNN_TIME") == "1":
        t0 = time.perf_counter_ns()
        res = runner(in_maps)
        LAST_EXEC_NS = time.perf_counter_ns() - t0

    ho = np.asarray(res[0]["hout"], np.float32)   # [128, 384]
    x_add = np.empty((G, HC), np.float32)
    x_max = np.empty((G, HC), np.float32)
    x_mean = np.empty((G, HC), np.float32)
    for i, arr in enumerate([x_add, x_max, x_mean]):
        blk = ho[:, i * 2 * G:(i + 1) * 2 * G].reshape(128, 2, G)
        arr[:, 0:128] = blk[:, 0, :].T
        arr[:, 128:256] = blk[:, 1, :].T

    h = np.concatenate([x_add, x_max, x_mean], axis=1).astype(np.float32)
    h = h @ np.asarray(m1w, np.float32) + np.asarray(m1b, np.float32)
    h = np.where(h >= 0, h, np.float32(pa) * h)
    lg = h @ np.asarray(m2w, np.float32) + np.asarray(m2b, np.float32)
    mx = lg.max(axis=1, keepdims=True)
    sh = lg - mx
    return (sh - np.log(np.exp(sh).sum(axis=1, keepdims=True))).astype(np.float32)


# revision 20
# speedup vs baseline: 9.8696x; 1.0118x over previous
"""Full 3-layer TransformerConv GNN on 8 Trainium2 cores.

Sharding: edges sorted by dst and partitioned into 8 contiguous dst-node
ranges (2500 nodes/core, padded to 2560).  Node projections (q/k/v/skip)
are computed replicated on every core into global DRAM tables; each core
runs segment-softmax message aggregation only for its 20 local 128-node
dst windows via one-hot scatter matmuls (PSUM-accumulated per window).
Pre-BN layer outputs are AllGathered (feat-major) between layers; BN
statistics are computed replicated from the gathered tensor.  Per-graph
sum/max/mean pooling happens on device; only the [64,768] pooled tensor
returns to the host, which applies the tiny MLP head.

All host->device payload travels in ONE packed fp16 tensor per core
(int16 index sections bitcast) to minimize axon-tunnel transfer time,
which dominates the dispatch wall clock.  Device compute stays fp32.

Self-contained: shapes hardcoded, sharding derived from the inputs.
"""
import math
import os
import time
import numpy as np

import jax
from jax.sharding import Mesh, PartitionSpec
from jax.experimental.shard_map import shard_map

from concourse import bacc, bass, tile, mybir, library_config

P = 8
N, E, F_IN, ED, G = 20000, 640000, 128, 4, 64
HC = 256
NLOC = N // P          # 2500
NWIN = 20              # 128-node dst windows per core
NPL = NWIN * 128       # 2560 padded local nodes
NT = P * NPL           # 20480 padded global nodes
EPS = 1e-5
F32 = mybir.dt.float32
F16 = mybir.dt.float16
I16 = mybir.dt.int16

LAST_EXEC_NS = None

_CACHE = {}


# ----------------------------------------------------------------- host pack
def _pack_edges(src, dst, edge_attr):
    order = np.argsort(dst, kind="stable")
    so = src[order].astype(np.int64)
    do = dst[order].astype(np.int64)
    eao = edge_attr[order].astype(np.float32)

    core = do // NLOC
    wloc = (do - core * NLOC) >> 7          # local window [0, NWIN)
    cw = np.zeros((P, NWIN), np.int64)
    np.add.at(cw, (core, wloc), 1)
    nblk = (-(-cw // 128)).max(axis=0)      # common blocks per window
    wstart = np.zeros(NWIN + 1, np.int64)
    wstart[1:] = np.cumsum(nblk)
    NB = int(wstart[-1])
    EP = NB * 128

    key = core * NWIN + wloc
    kcounts = np.bincount(key, minlength=P * NWIN)
    kstarts = np.zeros(P * NWIN, np.int64)
    kstarts[1:] = np.cumsum(kcounts)[:-1]
    ko = np.argsort(key, kind="stable")
    pos = np.empty(len(so), np.int64)
    pos[ko] = np.arange(len(so)) - kstarts[key[ko]]
    slot = wstart[wloc] * 128 + pos
    fi = core * EP + slot

    flat_ea = np.zeros((P * EP, ED), np.float16)
    flat_dl = np.full(P * EP, -1.0, np.float16)
    flat_si = np.zeros(P * EP, np.int64)
    flat_ea[fi] = eao
    flat_dl[fi] = (do - core * NLOC - wloc * 128).astype(np.float16)
    sc = so // NLOC
    flat_si[fi] = sc * NPL + (so - sc * NLOC)

    ea_l, dl_l, si_l = [], [], []
    for m in range(P):
        ea_l.append(np.ascontiguousarray(flat_ea[m * EP:(m + 1) * EP].T))
        dl_l.append(np.ascontiguousarray(
            flat_dl[m * EP:(m + 1) * EP].reshape(NB, 128).T))
        s16 = flat_si[m * EP:(m + 1) * EP].astype(np.int16)
        si_l.append(np.ascontiguousarray(s16.reshape(EP // 16, 16).T))
    return [int(v) for v in nblk], [int(v) for v in wstart], ea_l, dl_l, si_l


def _graph_segments(batch):
    gcnt = np.bincount(batch, minlength=G)
    assert (gcnt > 0).all(), "empty graph segment"
    gstart = np.zeros(G + 1, np.int64)
    gstart[1:] = np.cumsum(gcnt)
    parts = []  # per graph: list of (rank, lo, ln)
    for g in range(G):
        s, e = int(gstart[g]), int(gstart[g + 1])
        pl = []
        for r in range(s // NLOC, (e - 1) // NLOC + 1):
            lo = max(s, r * NLOC) - r * NLOC
            hi = min(e, (r + 1) * NLOC) - r * NLOC
            pl.append((r, lo, hi - lo))
        assert 1 <= len(pl) <= 2
        parts.append(pl)
    return parts, gcnt


def _wall_layout():
    """Replicated payload (weights etc), AllGathered on device from
    per-core 1/8 shards.  name -> (offset, nelem); 128-elem aligned."""
    sizes = [
        ("w1", F_IN * 4 * HC),
        ("w2", HC * 4 * HC),
        ("w3", HC * 4 * HC),
        ("b", 3 * 4 * HC),
        ("ew", ED * 3 * HC),
        ("bnp", 128 * 12),
        ("rc", 128),           # per-graph 1/count row (G used)
        ("iota", 128),         # row 0..127
    ]
    off, out = 0, {}
    for name, ne in sizes:
        out[name] = (off, ne)
        off += (ne + 127) // 128 * 128
    wall = (off + 8 * 128 - 1) // (8 * 128) * (8 * 128)
    return out, wall


def _sections(NB):
    """Packed-tensor layout: name -> (offset, nelem), 128-elem aligned."""
    EP = NB * 128
    _, WALL = _wall_layout()
    sizes = [
        ("x", F_IN * NPL),
        ("ea", ED * EP),
        ("dl", 128 * NB),
        ("si", EP),            # [16, EP/16] int16
        ("qi", NPL),           # [16, NPL/16] int16
        ("wsh", WALL // 8),    # this core's shard of the replicated wall
    ]
    off, out = 0, {}
    for name, ne in sizes:
        out[name] = (off, ne)
        off += (ne + 127) // 128 * 128
    return out, off


# -------------------------------------------------------------- bass program
def _build_program(nblk, wstart, NB, graph_parts):
    STOP = os.environ.get("BASS_GNN_STOP", "full")

    def _stopped(tag):
        order = ["l0proj", "l0edge", "l0post", "l1proj", "l1edge", "l1post",
                 "l2proj", "l2edge", "l2post", "pool", "full"]
        return order.index(STOP) < order.index(tag)

    EP = NB * 128
    SEC, TOT = _sections(NB)
    nc = bacc.Bacc("TRN2", debug=False, num_devices=P)

    PK = nc.dram_tensor("PK", [1, TOT], F16, kind="ExternalInput")
    hout = nc.dram_tensor("hout", [128, 3 * 2 * G], F16, kind="ExternalOutput")
    PKt = PK[:].tensor

    def sec2d(name, p, f, sub_off=0):
        if name in SECW:
            off, ne = SECW[name]
            assert p * f + sub_off <= ne
            return bass.AP(tensor=wbuf[:].tensor, offset=off + sub_off,
                           ap=[[f, p], [1, f]])
        off, ne = SEC[name]
        assert p * f + sub_off <= ne
        return bass.AP(tensor=PKt, offset=off + sub_off, ap=[[f, p], [1, f]])

    def wsec_off(name):
        return SECW[name][0]

    SECW, WALL = _wall_layout()
    wpart = nc.dram_tensor("wpart", [1, WALL // 8], F16, kind="Internal")
    wbuf = nc.dram_tensor("wbuf", [1, WALL], F16, kind="Internal",
                          addr_space="Shared")
    cc1 = nc.dram_tensor("cc1", [F_IN, NPL], F16, kind="Internal")
    xg1 = nc.dram_tensor("xg1", [P, F_IN, NPL], F16, kind="Internal",
                         addr_space="Shared")
    xg = [nc.dram_tensor(f"xg{l+2}", [P, HC, NPL], F32, kind="Internal",
                         addr_space="Shared") for l in range(3)]
    qtab = [nc.dram_tensor(f"qtab{l}", [NT, HC], F32, kind="Internal")
            for l in range(3)]
    ktab = [nc.dram_tensor(f"ktab{l}", [NT, HC], F32, kind="Internal")
            for l in range(3)]
    vtab = [nc.dram_tensor(f"vtab{l}", [NT, HC], F32, kind="Internal")
            for l in range(3)]
    stab = [nc.dram_tensor(f"stab{l}", [NT, HC], F32, kind="Internal")
            for l in range(3)]
    aggT = [nc.dram_tensor(f"aggT{l}", [HC, NPL], F32, kind="Internal")
            for l in range(3)]

    AG = mybir.AluOpType
    AF = mybir.ActivationFunctionType
    groups = [list(range(P))]
    holds = {}

    with tile.TileContext(nc) as tc:
        nc.gpsimd.load_library(library_config.mlp)
        nc.sync.dma_start(
            wpart[:],
            bass.AP(tensor=PKt, offset=SEC["wsh"][0],
                    ap=[[1, 1], [1, WALL // 8]]))
        nc.gpsimd.collective_compute(
            "AllGather", mybir.AluOpType.bypass, replica_groups=groups,
            ins=[wpart[:]], outs=[wbuf[:]])
        with tc.tile_pool(name="const", bufs=1) as cp:
            # iota row -> broadcast tiles, identity, tail mask
            o_iota = wsec_off("iota")
            io16 = cp.tile([128, 128], F16, name="io16")
            nc.gpsimd.dma_start(io16[:], bass.AP(
                tensor=wbuf[:].tensor, offset=o_iota, ap=[[0, 128], [1, 128]]))
            io32 = cp.tile([128, 128], F32, name="io32")
            nc.vector.tensor_copy(io32[:], io16[:])
            ioc16 = cp.tile([128, 1], F16, name="ioc16")
            nc.gpsimd.dma_start(ioc16[:], bass.AP(
                tensor=wbuf[:].tensor, offset=o_iota, ap=[[1, 128], [1, 1]]))
            ioc32 = cp.tile([128, 1], F32, name="ioc32")
            nc.vector.tensor_copy(ioc32[:], ioc16[:])
            s_eye = cp.tile([128, 128], F32, name="s_eye")
            nc.vector.tensor_scalar(s_eye[:], io32[:], ioc32[:], None,
                                    AG.is_equal)
            s_MASK = cp.tile([128, 1], F32, name="s_MASK")
            nc.vector.tensor_scalar(
                s_MASK[:], ioc32[:], float(NLOC - (NWIN - 1) * 128), None,
                AG.is_lt)

            # weights -> f32 SBUF
            s_W = []
            for l, (wn, K) in enumerate([("w1", F_IN), ("w2", HC), ("w3", HC)]):
                tiles = []
                for kh in range(K // 128):
                    t16 = cp.tile([128, 4 * HC], F16, name=f"w16_{l}_{kh}",
                                  tag="w16stage")
                    nc.sync.dma_start(
                        t16[:], sec2d(wn, 128, 4 * HC,
                                      sub_off=kh * 128 * 4 * HC))
                    t = cp.tile([128, 4 * HC], F32, name=f"s_W{l}_{kh}")
                    nc.vector.tensor_copy(t[:], t16[:])
                    tiles.append(t)
                s_W.append(tiles)
            s_B = []
            o_b = wsec_off("b")
            for l in range(3):
                t16 = cp.tile([128, 4 * HC], F16, name=f"b16_{l}",
                              tag="w16stage")
                nc.gpsimd.dma_start(t16[:], bass.AP(
                    tensor=wbuf[:].tensor, offset=o_b + l * 4 * HC,
                    ap=[[0, 128], [1, 4 * HC]]))
                t = cp.tile([128, 4 * HC], F32, name=f"s_B{l}")
                nc.vector.tensor_copy(t[:], t16[:])
                s_B.append(t)
            s_EW = cp.tile([ED, 3 * HC], F16, name="s_EW")
            nc.sync.dma_start(s_EW[:], sec2d("ew", ED, 3 * HC))
            bnp16 = cp.tile([128, 12], F16, name="bnp16")
            nc.sync.dma_start(bnp16[:], sec2d("bnp", 128, 12))
            s_BNP = cp.tile([128, 12], F32, name="s_BNP")
            nc.vector.tensor_copy(s_BNP[:], bnp16[:])
            rc16 = cp.tile([128, G], F16, name="rc16")
            nc.gpsimd.dma_start(rc16[:], bass.AP(
                tensor=wbuf[:].tensor, offset=wsec_off("rc"),
                ap=[[0, 128], [1, G]]))
            s_RC = cp.tile([128, G], F32, name="s_RC")
            nc.vector.tensor_copy(s_RC[:], rc16[:])

            dl16 = cp.tile([128, NB], F16, name="dl16")
            nc.sync.dma_start(dl16[:], sec2d("dl", 128, NB))
            s_dstl = cp.tile([128, NB], F32, name="s_dstl")
            nc.vector.tensor_copy(s_dstl[:], dl16[:])
            # int16 index sections: [16, C] on wire -> replicate to 128 rows
            C = EP // 16
            s_sidx = cp.tile([128, C], I16, name="s_sidx")
            nc.gpsimd.dma_start(s_sidx[:], bass.AP(
                tensor=PKt, offset=SEC["si"][0],
                ap=[[0, 8], [C, 16], [1, C]]).bitcast(I16))
            CQ = NPL // 16
            s_qidx = cp.tile([128, CQ], I16, name="s_qidx")
            nc.gpsimd.dma_start(s_qidx[:], bass.AP(
                tensor=PKt, offset=SEC["qi"][0],
                ap=[[0, 8], [CQ, 16], [1, CQ]]).bitcast(I16))

            # ------------------------------------------- x AllGather (layer 1)
            nc.sync.dma_start(
                cc1[:].rearrange("a b -> (a b)").unsqueeze(0),
                bass.AP(tensor=PKt, offset=SEC["x"][0],
                        ap=[[1, 1], [1, F_IN * NPL]]))
            nc.gpsimd.collective_compute(
                "AllGather", AG.bypass, replica_groups=groups,
                ins=[cc1[:]], outs=[xg1[:]])

            sbn = [None, None, None]
            tbn = [None, None, None]
            sbn3 = tbn3 = None

            # ======================================================= layers
            for l in range(3):
                if _stopped(f"l{l}proj"):
                    break
                H = 4 if l == 0 else 1
                C_h = 64 if l == 0 else HC
                inv_sqrt_c = 1.0 / math.sqrt(C_h)
                K = F_IN if l == 0 else HC
                src_g = xg1 if l == 0 else xg[l - 1]

                # ---- projections: q/k/v/s tables for all NT nodes
                with (tc.tile_pool(name=f"pj{l}", bufs=3) as wp,
                      tc.tile_pool(name=f"pjp{l}", bufs=2, space="PSUM") as pp):
                    for r in range(P):
                        for c in range(NPL // 512):
                            xts = []
                            for kh in range(K // 128):
                                if l == 0:
                                    x16 = wp.tile([128, 512], F16,
                                                  tag=f"x16_{kh}")
                                    nc.sync.dma_start(
                                        x16[:],
                                        src_g[r, kh * 128:(kh + 1) * 128,
                                              c * 512:(c + 1) * 512])
                                    xb = wp.tile([128, 512], F32,
                                                 tag=f"xb{kh}")
                                    nc.vector.tensor_copy(xb[:], x16[:])
                                    xts.append(xb)
                                else:
                                    xt = wp.tile([128, 512], F32,
                                                 tag=f"xt{kh}")
                                    nc.sync.dma_start(
                                        xt[:],
                                        src_g[r, kh * 128:(kh + 1) * 128,
                                              c * 512:(c + 1) * 512])
                                    xb = wp.tile([128, 512], F32,
                                                 tag=f"xb{kh}")
                                    nc.scalar.activation(
                                        xb[:], xt[:], AF.Identity,
                                        bias=tbn[l][:, kh:kh + 1],
                                        scale=sbn[l][:, kh:kh + 1])
                                    xts.append(xb)
                            for sub in range(4):
                                row0 = r * NPL + c * 512 + sub * 128
                                for half in range(2):
                                    ppt = pp.tile(
                                        [128, 512], F32, tag=f"pp{half}",
                                        name=f"pp{l}_{r}_{c}_{sub}_{half}")
                                    nkh = K // 128
                                    for kh in range(nkh):
                                        nc.tensor.matmul(
                                            ppt[:],
                                            xts[kh][:, sub * 128:(sub + 1) * 128],
                                            s_W[l][kh][:, half * 512:(half + 1) * 512],
                                            start=(kh == 0),
                                            stop=(kh == nkh - 1))
                                    ob = wp.tile([128, 512], F32, tag="ob")
                                    nc.vector.tensor_tensor(
                                        ob[:], ppt[:],
                                        s_B[l][:, half * 512:(half + 1) * 512],
                                        AG.add)
                                    if half == 0:
                                        nc.sync.dma_start(
                                            qtab[l][row0:row0 + 128, :],
                                            ob[:, 0:HC])
                                        nc.sync.dma_start(
                                            ktab[l][row0:row0 + 128, :],
                                            ob[:, HC:2 * HC])
                                    else:
                                        nc.sync.dma_start(
                                            vtab[l][row0:row0 + 128, :],
                                            ob[:, 0:HC])
                                        nc.sync.dma_start(
                                            stab[l][row0:row0 + 128, :],
                                            ob[:, HC:2 * HC])

                # ---- edge phase: per-window segment softmax + aggregation
                if _stopped(f"l{l}edge"):
                    break
                ewsl = s_EW[:, l * HC:(l + 1) * HC]
                with (tc.tile_pool(name=f"ed{l}", bufs=3) as wp,
                      tc.tile_pool(name=f"edp{l}", bufs=2, space="PSUM") as pe):
                    for w in range(NWIN):
                        qwg = wp.tile([128, 1, HC], F32, tag="qw")
                        nc.gpsimd.dma_gather(
                            qwg[:], qtab[l][:], s_qidx[:, w * 8:(w + 1) * 8],
                            128, 128, HC)
                        swg = wp.tile([128, 1, HC], F32, tag="sw")
                        nc.gpsimd.dma_gather(
                            swg[:], stab[l][:], s_qidx[:, w * 8:(w + 1) * 8],
                            128, 128, HC)
                        qw = qwg[:].rearrange("p a f -> p (a f)")
                        sw = swg[:].rearrange("p a f -> p (a f)")
                        nb = nblk[w]
                        pagg = (pe.tile([128, HC + H], F32, tag="agg",
                                        name=f"pagg{l}_{w}")
                                if nb > 0 else None)
                        for b in range(nb):
                            blk = wstart[w] + b
                            S = wp.tile([128, 128], F32, tag="S")
                            nc.vector.tensor_scalar(
                                S[:], io32[:], s_dstl[:, blk:blk + 1], None,
                                AG.is_equal)
                            pS = pe.tile([128, 128], F32, tag="pS")
                            nc.tensor.transpose(pS[:], S[:], s_eye[:])
                            ST = wp.tile([128, 128], F32, tag="ST")
                            nc.scalar.copy(ST[:], pS[:])
                            kg = wp.tile([128, 1, HC], F32, tag="kg")
                            nc.gpsimd.dma_gather(
                                kg[:], ktab[l][:],
                                s_sidx[:, blk * 8:(blk + 1) * 8], 128, 128, HC)
                            vg = wp.tile([128, 1, HC], F32, tag="vg")
                            nc.gpsimd.dma_gather(
                                vg[:], vtab[l][:],
                                s_sidx[:, blk * 8:(blk + 1) * 8], 128, 128, HC)
                            kg2 = kg[:].rearrange("p a f -> p (a f)")
                            vg2 = vg[:].rearrange("p a f -> p (a f)")
                            eat = wp.tile([ED, 128], F16, tag="eat")
                            nc.sync.dma_start(
                                eat[:], sec2d("ea", ED, EP)[
                                    :, blk * 128:(blk + 1) * 128])
                            pE = pe.tile([128, HC], F32, tag="pE")
                            nc.tensor.matmul(pE[:], eat[:], ewsl,
                                             start=True, stop=True)
                            pQ = pe.tile([128, HC], F32, tag="pQ")
                            nc.tensor.matmul(pQ[:], ST[:], qw,
                                             start=True, stop=True)
                            kj = wp.tile([128, HC], F32, tag="kj")
                            nc.vector.tensor_tensor(kj[:], kg2, pE[:], AG.add)
                            prod = wp.tile([128, HC], F32, tag="prod")
                            nc.vector.tensor_tensor(prod[:], pQ[:], kj[:],
                                                    AG.mult)
                            al = wp.tile([128, H], F32, tag="al")
                            nc.vector.tensor_reduce(
                                al[:],
                                prod[:].rearrange("p (h c) -> p h c", h=H),
                                mybir.AxisListType.X, AG.add)
                            ex = wp.tile([128, H], F32, tag="ex")
                            nc.scalar.activation(ex[:], al[:], AF.Exp,
                                                 scale=inv_sqrt_c)
                            vj = wp.tile([128, HC], F32, tag="vj")
                            nc.vector.tensor_tensor(vj[:], vg2, pE[:], AG.add)
                            mv = wp.tile([128, HC + H], F32, tag="mv")
                            if H == 1:
                                nc.vector.tensor_scalar_mul(mv[:, 0:HC], vj[:],
                                                            ex[:, 0:1])
                            else:
                                nc.vector.tensor_tensor(
                                    mv[:, 0:HC].rearrange(
                                        "p (h c) -> p h c", h=H),
                                    vj[:].rearrange("p (h c) -> p h c", h=H),
                                    ex[:].unsqueeze(2).broadcast_to(
                                        [128, H, C_h]),
                                    AG.mult)
                            nc.vector.tensor_copy(mv[:, HC:HC + H], ex[:])
                            nc.tensor.matmul(pagg[:, 0:HC + H], S[:], mv[:],
                                             start=(b == 0),
                                             stop=(b == nb - 1))
                        # window evacuation
                        ob2 = wp.tile([128, HC], F32, tag="ob2")
                        if nb == 0:
                            nc.vector.tensor_copy(ob2[:], sw)
                        else:
                            den = wp.tile([128, H], F32, tag="den")
                            nc.vector.tensor_scalar_add(
                                den[:], pagg[:, HC:HC + H], 1e-16)
                            rc = wp.tile([128, H], F32, tag="rc")
                            nc.vector.reciprocal(rc[:], den[:])
                            ob = wp.tile([128, HC], F32, tag="ob")
                            if H == 1:
                                nc.vector.tensor_scalar_mul(
                                    ob[:], pagg[:, 0:HC], rc[:, 0:1])
                            else:
                                nc.vector.tensor_tensor(
                                    ob[:].rearrange("p (h c) -> p h c", h=H),
                                    pagg[:, 0:HC].rearrange(
                                        "p (h c) -> p h c", h=H),
                                    rc[:].unsqueeze(2).broadcast_to(
                                        [128, H, C_h]),
                                    AG.mult)
                            nc.vector.tensor_tensor(ob2[:], ob[:], sw, AG.add)
                        if w == NWIN - 1:
                            obm = wp.tile([128, HC], F32, tag="obm")
                            nc.vector.tensor_scalar_mul(obm[:], ob2[:],
                                                        s_MASK[:, 0:1])
                        else:
                            obm = ob2
                        for fh in range(2):
                            pt = pe.tile([128, 128], F32, tag="pS",
                                         name=f"pt{l}_{w}_{fh}")
                            nc.tensor.transpose(
                                pt[:], obm[:, fh * 128:(fh + 1) * 128],
                                s_eye[:])
                            tb = wp.tile([128, 128], F32, tag="tb")
                            nc.vector.tensor_copy(tb[:], pt[:])
                            nc.sync.dma_start(
                                aggT[l][fh * 128:(fh + 1) * 128,
                                        w * 128:(w + 1) * 128], tb[:])

                # ---- AllGather pre-BN output, then global BN stats
                if _stopped(f"l{l}post"):
                    break
                nc.gpsimd.collective_compute(
                    "AllGather", AG.bypass, replica_groups=groups,
                    ins=[aggT[l][:]], outs=[xg[l][:]])

                with tc.tile_pool(name=f"st{l}", bufs=2) as wp:
                    sums = wp.tile([128, 2, P], F32, tag="sums")
                    sqs = wp.tile([128, 2, P], F32, tag="sqs")
                    for r in range(P):
                        for fh in range(2):
                            ch = wp.tile([128, NPL], F32, tag="ch")
                            nc.sync.dma_start(
                                ch[:], xg[l][r, fh * 128:(fh + 1) * 128, :])
                            nc.vector.tensor_reduce(
                                sums[:, fh:fh + 1, r:r + 1].rearrange(
                                    "p a b -> p (a b)"),
                                ch[:], mybir.AxisListType.X, AG.add)
                            scr = wp.tile([128, NPL], F32, tag="scr")
                            nc.scalar.activation(
                                scr[:], ch[:], AF.Square,
                                accum_out=sqs[:, fh:fh + 1, r:r + 1].rearrange(
                                    "p a b -> p (a b)"))
                    musum = wp.tile([128, 2], F32, tag="musum")
                    nc.vector.tensor_reduce(musum[:], sums[:],
                                            mybir.AxisListType.X, AG.add)
                    mu = cp.tile([128, 2], F32, name=f"mu{l}")
                    nc.vector.tensor_scalar_mul(mu[:], musum[:], 1.0 / N)
                    sqsum = wp.tile([128, 2], F32, tag="sqsum")
                    nc.vector.tensor_reduce(sqsum[:], sqs[:],
                                            mybir.AxisListType.X, AG.add)
                    ex2 = wp.tile([128, 2], F32, tag="ex2")
                    nc.vector.tensor_scalar_mul(ex2[:], sqsum[:], 1.0 / N)
                    m2 = wp.tile([128, 2], F32, tag="m2")
                    nc.vector.tensor_tensor(m2[:], mu[:], mu[:], AG.mult)
                    var = wp.tile([128, 2], F32, tag="var")
                    nc.vector.tensor_tensor(var[:], ex2[:], m2[:], AG.subtract)
                    vpe = wp.tile([128, 2], F32, tag="vpe")
                    nc.vector.tensor_scalar_add(vpe[:], var[:], EPS)
                    sd = wp.tile([128, 2], F32, tag="sd")
                    nc.scalar.activation(sd[:], vpe[:], AF.Sqrt)
                    rstd = wp.tile([128, 2], F32, tag="rstd")
                    nc.vector.reciprocal(rstd[:], sd[:])
                    sb_t = cp.tile([128, 2], F32, name=f"sbn{l}")
                    nc.vector.tensor_tensor(sb_t[:], rstd[:],
                                            s_BNP[:, 2 * l:2 * l + 2], AG.mult)
                    tmp = wp.tile([128, 2], F32, tag="tmp")
                    nc.vector.tensor_tensor(tmp[:], mu[:], sb_t[:], AG.mult)
                    tb_t = cp.tile([128, 2], F32, name=f"tbn{l}")
                    nc.vector.tensor_tensor(
                        tb_t[:], s_BNP[:, 6 + 2 * l:8 + 2 * l], tmp[:],
                        AG.subtract)
                    if l < 2:
                        sbn[l + 1] = sb_t
                        tbn[l + 1] = tb_t
                    else:
                        sbn3, tbn3 = sb_t, tb_t

            # =============================================== pooling (layer 3)
            with tc.tile_pool(name="pool", bufs=2) as wp:
              if not _stopped("pool"):
                padd = wp.tile([128, 2, G], F32, tag="padd")
                pmax = wp.tile([128, 2, G], F32, tag="pmax")
                for fh in range(2):
                    for r in range(P):
                        ch = wp.tile([128, NLOC], F32, tag="pch")
                        nc.sync.dma_start(
                            ch[:], xg[2][r, fh * 128:(fh + 1) * 128, 0:NLOC])
                        bnc = wp.tile([128, NLOC], F32, tag="pbn")
                        nc.scalar.activation(bnc[:], ch[:], AF.Identity,
                                             bias=tbn3[:, fh:fh + 1],
                                             scale=sbn3[:, fh:fh + 1])
                        for g, pl in enumerate(graph_parts):
                            for (pr, lo, ln) in pl:
                                if pr != r:
                                    continue
                                seg = bnc[:, lo:lo + ln]
                                if len(pl) == 1:
                                    nc.vector.tensor_reduce(
                                        padd[:, fh:fh + 1, g:g + 1].rearrange(
                                            "p a b -> p (a b)"),
                                        seg, mybir.AxisListType.X, AG.add)
                                    nc.vector.tensor_reduce(
                                        pmax[:, fh:fh + 1, g:g + 1].rearrange(
                                            "p a b -> p (a b)"),
                                        seg, mybir.AxisListType.X, AG.max)
                                else:
                                    first = (pr, lo, ln) == pl[0]
                                    sfx = "a" if first else "b"
                                    ta = wp.tile([128, 1], F32,
                                                 tag=f"t{sfx}_add",
                                                 name=f"t{sfx}a_{fh}_{g}")
                                    nc.vector.tensor_reduce(
                                        ta[:], seg, mybir.AxisListType.X,
                                        AG.add)
                                    tm = wp.tile([128, 1], F32,
                                                 tag=f"t{sfx}_max",
                                                 name=f"t{sfx}m_{fh}_{g}")
                                    nc.vector.tensor_reduce(
                                        tm[:], seg, mybir.AxisListType.X,
                                        AG.max)
                                    if first:
                                        holds[(fh, g)] = (ta, tm)
                                    else:
                                        ha, hm = holds.pop((fh, g))
                                        nc.vector.tensor_tensor(
                                            padd[:, fh:fh + 1,
                                                 g:g + 1].rearrange(
                                                "p a b -> p (a b)"),
                                            ha[:], ta[:], AG.add)
                                        nc.vector.tensor_tensor(
                                            pmax[:, fh:fh + 1,
                                                 g:g + 1].rearrange(
                                                "p a b -> p (a b)"),
                                            hm[:], tm[:], AG.max)
                pmean = wp.tile([128, 2, G], F32, tag="pmean")
                for fh in range(2):
                    nc.vector.tensor_tensor(
                        pmean[:, fh, :], padd[:, fh, :], s_RC[:], AG.mult)
                h16 = wp.tile([128, 3 * 2 * G], F16, tag="h16")
                nc.vector.tensor_copy(
                    h16[:, 0:2 * G], padd[:].rearrange("p a g -> p (a g)"))
                nc.vector.tensor_copy(
                    h16[:, 2 * G:4 * G], pmax[:].rearrange("p a g -> p (a g)"))
                nc.vector.tensor_copy(
                    h16[:, 4 * G:6 * G], pmean[:].rearrange("p a g -> p (a g)"))
                nc.sync.dma_start(hout[:], h16[:])
    nc.finalize()
    return nc


# ---------------------------------------------------------------- jit runner
class _Runner:
    """Build the PJRT executable once; each call = H2D + execute + D2H."""

    def __init__(self, nc, n_cores):
        from concourse.bass2jax import (install_neuronx_cc_hook, _bass_exec_p,
                                        partition_id_tensor)
        install_neuronx_cc_hook()
        self.nc = nc
        partition_name = (nc.partition_id_tensor.name
                          if nc.partition_id_tensor else None)
        in_names, out_names, out_avals, zero_shapes = [], [], [], []
        for alloc in nc.m.functions[0].allocations:
            if not isinstance(alloc, mybir.MemoryLocationSet):
                continue
            name = alloc.memorylocations[0].name
            if alloc.kind == "ExternalInput":
                if name != partition_name:
                    in_names.append(name)
            elif alloc.kind == "ExternalOutput":
                out_names.append(name)
                shape = tuple(alloc.tensor_shape)
                dtype = mybir.dt.np(alloc.dtype)
                out_avals.append(jax.core.ShapedArray(shape, dtype))
                zero_shapes.append((shape, dtype))
        self.in_names, self.out_names = in_names, out_names
        self.zero_shapes = zero_shapes
        n_params, n_outs = len(in_names), len(out_avals)
        all_names = (list(in_names) + list(out_names)
                     + ([partition_name] if partition_name else []))

        def _body(*args):
            operands = list(args)
            if partition_name is not None:
                operands.append(partition_id_tensor())
            outs = _bass_exec_p.bind(
                *operands, out_avals=tuple(out_avals),
                in_names=tuple(all_names), out_names=tuple(out_names),
                lowering_input_output_aliases=(), sim_require_finite=True,
                sim_require_nnan=True, nc=nc)
            return tuple(outs)

        devices = jax.devices()[:n_cores]
        mesh = Mesh(np.asarray(devices), ("core",))
        in_specs = (PartitionSpec("core"),) * (n_params + n_outs)
        out_specs = (PartitionSpec("core"),) * n_outs
        self.n_cores = n_cores
        self.fn = jax.jit(
            shard_map(_body, mesh=mesh, in_specs=in_specs,
                      out_specs=out_specs, check_rep=False),
            donate_argnums=tuple(range(n_params, n_params + n_outs)),
            keep_unused=True)

    def __call__(self, in_maps):
        if getattr(self, "_cin_src", None) is in_maps:
            concat_in = self._cin
        else:
            per_core = [[np.asarray(m[n]) for n in self.in_names]
                        for m in in_maps]
            concat_in = [np.concatenate(
                [per_core[c][i] for c in range(self.n_cores)], axis=0)
                for i in range(len(self.in_names))]
            self._cin_src, self._cin = in_maps, concat_in
        zeros = [np.zeros((self.n_cores * s[0], *s[1:]), d)
                 for s, d in self.zero_shapes]
        outs = self.fn(*concat_in, *zeros)
        outs = [np.asarray(o) for o in outs]
        return [{n: outs[i].reshape(self.n_cores, *self.zero_shapes[i][0])[c]
                 for i, n in enumerate(self.out_names)}
                for c in range(self.n_cores)]


# --------------------------------------------------------------------- kernel
def kernel(x, edge_index, edge_attr, batch,
           q1w, q1b, k1w, k1b, v1w, v1b, e1w, s1w, s1b, bn1w, bn1b,
           q2w, q2b, k2w, k2b, v2w, v2b, e2w, s2w, s2b, bn2w, bn2b,
           q3w, q3b, k3w, k3b, v3w, v3b, e3w, s3w, s3b, bn3w, bn3b,
           m1w, m1b, pa, m2w, m2b):
    global LAST_EXEC_NS
    x = np.asarray(x, np.float32)
    edge_index = np.asarray(edge_index)
    edge_attr = np.asarray(edge_attr, np.float32)
    batch = np.asarray(batch)
    src, dst = edge_index[0], edge_index[1]

    nblk, wstart, ea_l, dl_l, si_l = _pack_edges(src, dst, edge_attr)
    NB = wstart[-1]
    graph_parts, gcnt = _graph_segments(batch)
    SEC, TOT = _sections(NB)

    key = (tuple(nblk), tuple(tuple(p) for pl in graph_parts for p in pl))
    if key in _CACHE:
        runner = _CACHE[key]
    else:
        nc = _build_program(nblk, wstart, NB, graph_parts)
        runner = _Runner(nc, P)
        _CACHE[key] = runner

    xp = np.zeros((NT, F_IN), np.float16)
    for m in range(P):
        xp[m * NPL:m * NPL + NLOC] = x[m * NLOC:(m + 1) * NLOC]

    def f16(a):
        return np.asarray(a, np.float16)

    com = {}   # replicated sections, flat f16
    com["w1"] = f16(np.hstack([q1w, k1w, v1w, s1w])).ravel()
    com["w2"] = f16(np.hstack([q2w, k2w, v2w, s2w])).ravel()
    com["w3"] = f16(np.hstack([q3w, k3w, v3w, s3w])).ravel()
    com["b"] = f16(np.concatenate(
        [np.hstack([q1b, k1b, v1b, s1b]), np.hstack([q2b, k2b, v2b, s2b]),
         np.hstack([q3b, k3b, v3b, s3b])]))
    com["ew"] = f16(np.hstack([e1w, e2w, e3w])).ravel()
    bnp = np.zeros((128, 12), np.float16)
    for l, (bw, bb) in enumerate([(bn1w, bn1b), (bn2w, bn2b), (bn3w, bn3b)]):
        bnp[:, 2 * l:2 * l + 2] = np.asarray(bw).reshape(2, 128).T
        bnp[:, 6 + 2 * l:8 + 2 * l] = np.asarray(bb).reshape(2, 128).T
    com["bnp"] = bnp.ravel()
    rcv = np.zeros(128, np.float16)
    rcv[0:G] = (1.0 / np.maximum(gcnt, 1)).astype(np.float16)
    com["rc"] = rcv
    com["iota"] = np.arange(128, dtype=np.float16)

    SECW, WALL = _wall_layout()
    wall = np.zeros(WALL, np.float16)
    for name, arr in com.items():
        o, ne = SECW[name]
        wall[o:o + len(arr)] = arr
    WSH = WALL // 8

    in_maps = []
    for m in range(P):
        pk = np.zeros(TOT, np.float16)
        o, ne = SEC["x"]
        pk[o:o + ne] = np.ascontiguousarray(
            xp[m * NPL:(m + 1) * NPL].T).ravel()
        o, ne = SEC["ea"]
        pk[o:o + ne] = ea_l[m].ravel()
        o, ne = SEC["dl"]
        pk[o:o + ne] = dl_l[m].ravel()
        o, ne = SEC["si"]
        pk[o:o + ne] = si_l[m].ravel().view(np.float16)
        o, ne = SEC["qi"]
        ids = (m * NPL + np.arange(NPL)).astype(np.int16)
        pk[o:o + ne] = np.ascontiguousarray(
            ids.reshape(NPL // 16, 16).T).ravel().view(np.float16)
        o, ne = SEC["wsh"]
        pk[o:o + WSH] = wall[m * WSH:(m + 1) * WSH]
        in_maps.append({"PK": pk[None, :]})

    res = runner(in_maps)
    if os.environ.get("BASS_GNN_TIME") == "1":
        t0 = time.perf_counter_ns()
        res = runner(in_maps)
        LAST_EXEC_NS = time.perf_counter_ns() - t0

    ho = np.asarray(res[0]["hout"], np.float32)   # [128, 384]
    x_add = np.empty((G, HC), np.float32)
    x_max = np.empty((G, HC), np.float32)
    x_mean = np.empty((G, HC), np.float32)
    for i, arr in enumerate([x_add, x_max, x_mean]):
        blk = ho[:, i * 2 * G:(i + 1) * 2 * G].reshape(128, 2, G)
        arr[:, 0:128] = blk[:, 0, :].T
        arr[:, 128:256] = blk[:, 1, :].T

    h = np.concatenate([x_add, x_max, x_mean], axis=1).astype(np.float32)
    h = h @ np.asarray(m1w, np.float32) + np.asarray(m1b, np.float32)
    h = np.where(h >= 0, h, np.float32(pa) * h)
    lg = h @ np.asarray(m2w, np.float32) + np.asarray(m2b, np.float32)
    mx = lg.max(axis=1, keepdims=True)
    sh = lg - mx
    return (sh - np.log(np.exp(sh).sum(axis=1, keepdims=True))).astype(np.float32)


# revision 21
# speedup vs baseline: 10.2560x; 1.0392x over previous
"""Full 3-layer TransformerConv GNN on 8 Trainium2 cores.

Sharding: edges sorted by dst and partitioned into 8 contiguous dst-node
ranges (2500 nodes/core, padded to 2560).  Node projections (q/k/v/skip)
are computed replicated on every core into global DRAM tables; each core
runs segment-softmax message aggregation only for its 20 local 128-node
dst windows via one-hot scatter matmuls (PSUM-accumulated per window).
Pre-BN layer outputs are AllGathered (feat-major) between layers; BN
statistics are computed replicated from the gathered tensor.  Per-graph
sum/max/mean pooling happens on device; only the [64,768] pooled tensor
returns to the host, which applies the tiny MLP head.

All host->device payload travels in ONE packed fp16 tensor per core
(int16 index sections bitcast) to minimize axon-tunnel transfer time,
which dominates the dispatch wall clock.  Device compute stays fp32.

Self-contained: shapes hardcoded, sharding derived from the inputs.
"""
import math
import os
import time
import numpy as np

import jax
from jax.sharding import Mesh, PartitionSpec
from jax.experimental.shard_map import shard_map

from concourse import bacc, bass, tile, mybir, library_config

P = 8
N, E, F_IN, ED, G = 20000, 640000, 128, 4, 64
HC = 256
NLOC = N // P          # 2500
NWIN = 20              # 128-node dst windows per core
NPL = NWIN * 128       # 2560 padded local nodes
NT = P * NPL           # 20480 padded global nodes
EPS = 1e-5
F32 = mybir.dt.float32
F16 = mybir.dt.float16
I16 = mybir.dt.int16

LAST_EXEC_NS = None

_CACHE = {}


# ----------------------------------------------------------------- host pack
def _pack_edges(src, dst, edge_attr):
    order = np.argsort(dst, kind="stable")
    so = src[order].astype(np.int64)
    do = dst[order].astype(np.int64)
    eao = edge_attr[order].astype(np.float32)

    core = do // NLOC
    wloc = (do - core * NLOC) >> 7          # local window [0, NWIN)
    cw = np.zeros((P, NWIN), np.int64)
    np.add.at(cw, (core, wloc), 1)
    nblk = (-(-cw // 128)).max(axis=0)      # common blocks per window
    wstart = np.zeros(NWIN + 1, np.int64)
    wstart[1:] = np.cumsum(nblk)
    NB = int(wstart[-1])
    EP = NB * 128

    key = core * NWIN + wloc
    kcounts = np.bincount(key, minlength=P * NWIN)
    kstarts = np.zeros(P * NWIN, np.int64)
    kstarts[1:] = np.cumsum(kcounts)[:-1]
    ko = np.argsort(key, kind="stable")
    pos = np.empty(len(so), np.int64)
    pos[ko] = np.arange(len(so)) - kstarts[key[ko]]
    slot = wstart[wloc] * 128 + pos
    fi = core * EP + slot

    flat_ea = np.zeros((P * EP, ED), np.float16)
    flat_dl = np.full(P * EP, -1, np.int8)
    flat_si = np.zeros(P * EP, np.int64)
    flat_ea[fi] = eao
    flat_dl[fi] = (do - core * NLOC - wloc * 128).astype(np.int8)
    sc = so // NLOC
    flat_si[fi] = sc * NPL + (so - sc * NLOC)

    ea_l, dl_l, si_l = [], [], []
    for m in range(P):
        ea_l.append(np.ascontiguousarray(flat_ea[m * EP:(m + 1) * EP].T))
        NBP = NB + NB % 2
        dlp = np.full((128, NBP), -1, np.int8)
        dlp[:, 0:NB] = flat_dl[m * EP:(m + 1) * EP].reshape(NB, 128).T
        dl_l.append(dlp)
        s16 = flat_si[m * EP:(m + 1) * EP].astype(np.int16)
        si_l.append(np.ascontiguousarray(s16.reshape(EP // 16, 16).T))
    return [int(v) for v in nblk], [int(v) for v in wstart], ea_l, dl_l, si_l


def _graph_segments(batch):
    gcnt = np.bincount(batch, minlength=G)
    assert (gcnt > 0).all(), "empty graph segment"
    gstart = np.zeros(G + 1, np.int64)
    gstart[1:] = np.cumsum(gcnt)
    parts = []  # per graph: list of (rank, lo, ln)
    for g in range(G):
        s, e = int(gstart[g]), int(gstart[g + 1])
        pl = []
        for r in range(s // NLOC, (e - 1) // NLOC + 1):
            lo = max(s, r * NLOC) - r * NLOC
            hi = min(e, (r + 1) * NLOC) - r * NLOC
            pl.append((r, lo, hi - lo))
        assert 1 <= len(pl) <= 2
        parts.append(pl)
    return parts, gcnt


def _wall_layout():
    """Replicated payload (weights etc), AllGathered on device from
    per-core 1/8 shards.  name -> (offset, nelem); 128-elem aligned."""
    sizes = [
        ("w1", F_IN * 4 * HC),
        ("w2", HC * 4 * HC),
        ("w3", HC * 4 * HC),
        ("b", 3 * 4 * HC),
        ("ew", ED * 3 * HC),
        ("bnp", 128 * 12),
        ("rc", 128),           # per-graph 1/count row (G used)
        ("iota", 128),         # row 0..127
    ]
    off, out = 0, {}
    for name, ne in sizes:
        out[name] = (off, ne)
        off += (ne + 127) // 128 * 128
    wall = (off + 8 * 128 - 1) // (8 * 128) * (8 * 128)
    return out, wall


def _sections(NB):
    """Packed-tensor layout: name -> (offset, nelem), 128-elem aligned."""
    EP = NB * 128
    _, WALL = _wall_layout()
    sizes = [
        ("x", F_IN * NPL),
        ("ea", ED * EP),
        ("dl", 64 * (NB + NB % 2)),   # [128, NBP] int8 rows
        ("si", EP),            # [16, EP/16] int16
        ("qi", NPL),           # [16, NPL/16] int16
        ("wsh", WALL // 8),    # this core's shard of the replicated wall
    ]
    off, out = 0, {}
    for name, ne in sizes:
        out[name] = (off, ne)
        off += (ne + 127) // 128 * 128
    return out, off


# -------------------------------------------------------------- bass program
def _build_program(nblk, wstart, NB, graph_parts):
    STOP = os.environ.get("BASS_GNN_STOP", "full")

    def _stopped(tag):
        order = ["l0proj", "l0edge", "l0post", "l1proj", "l1edge", "l1post",
                 "l2proj", "l2edge", "l2post", "pool", "full"]
        return order.index(STOP) < order.index(tag)

    EP = NB * 128
    SEC, TOT = _sections(NB)
    nc = bacc.Bacc("TRN2", debug=False, num_devices=P)

    PK = nc.dram_tensor("PK", [1, TOT], F16, kind="ExternalInput")
    hout = nc.dram_tensor("hout", [128, 3 * 2 * G], F16, kind="ExternalOutput")
    PKt = PK[:].tensor

    def sec2d(name, p, f, sub_off=0):
        if name in SECW:
            off, ne = SECW[name]
            assert p * f + sub_off <= ne
            return bass.AP(tensor=wbuf[:].tensor, offset=off + sub_off,
                           ap=[[f, p], [1, f]])
        off, ne = SEC[name]
        assert p * f + sub_off <= ne
        return bass.AP(tensor=PKt, offset=off + sub_off, ap=[[f, p], [1, f]])

    def wsec_off(name):
        return SECW[name][0]

    SECW, WALL = _wall_layout()
    wpart = nc.dram_tensor("wpart", [1, WALL // 8], F16, kind="Internal")
    wbuf = nc.dram_tensor("wbuf", [1, WALL], F16, kind="Internal",
                          addr_space="Shared")
    cc1 = nc.dram_tensor("cc1", [F_IN, NPL], F16, kind="Internal")
    xg1 = nc.dram_tensor("xg1", [P, F_IN, NPL], F16, kind="Internal",
                         addr_space="Shared")
    xg = [nc.dram_tensor(f"xg{l+2}", [P, HC, NPL], F32, kind="Internal",
                         addr_space="Shared") for l in range(3)]
    qtab = [nc.dram_tensor(f"qtab{l}", [NT, HC], F32, kind="Internal")
            for l in range(3)]
    ktab = [nc.dram_tensor(f"ktab{l}", [NT, HC], F32, kind="Internal")
            for l in range(3)]
    vtab = [nc.dram_tensor(f"vtab{l}", [NT, HC], F32, kind="Internal")
            for l in range(3)]
    stab = [nc.dram_tensor(f"stab{l}", [NT, HC], F32, kind="Internal")
            for l in range(3)]
    aggT = [nc.dram_tensor(f"aggT{l}", [HC, NPL], F32, kind="Internal")
            for l in range(3)]

    AG = mybir.AluOpType
    AF = mybir.ActivationFunctionType
    groups = [list(range(P))]
    holds = {}

    with tile.TileContext(nc) as tc:
        nc.gpsimd.load_library(library_config.mlp)
        nc.sync.dma_start(
            wpart[:],
            bass.AP(tensor=PKt, offset=SEC["wsh"][0],
                    ap=[[1, 1], [1, WALL // 8]]))
        nc.gpsimd.collective_compute(
            "AllGather", mybir.AluOpType.bypass, replica_groups=groups,
            ins=[wpart[:]], outs=[wbuf[:]])
        with tc.tile_pool(name="const", bufs=1) as cp:
            # iota row -> broadcast tiles, identity, tail mask
            o_iota = wsec_off("iota")
            io16 = cp.tile([128, 128], F16, name="io16")
            nc.gpsimd.dma_start(io16[:], bass.AP(
                tensor=wbuf[:].tensor, offset=o_iota, ap=[[0, 128], [1, 128]]))
            io32 = cp.tile([128, 128], F32, name="io32")
            nc.vector.tensor_copy(io32[:], io16[:])
            ioc16 = cp.tile([128, 1], F16, name="ioc16")
            nc.gpsimd.dma_start(ioc16[:], bass.AP(
                tensor=wbuf[:].tensor, offset=o_iota, ap=[[1, 128], [1, 1]]))
            ioc32 = cp.tile([128, 1], F32, name="ioc32")
            nc.vector.tensor_copy(ioc32[:], ioc16[:])
            s_eye = cp.tile([128, 128], F32, name="s_eye")
            nc.vector.tensor_scalar(s_eye[:], io32[:], ioc32[:], None,
                                    AG.is_equal)
            s_MASK = cp.tile([128, 1], F32, name="s_MASK")
            nc.vector.tensor_scalar(
                s_MASK[:], ioc32[:], float(NLOC - (NWIN - 1) * 128), None,
                AG.is_lt)

            # weights -> f32 SBUF
            s_W = []
            for l, (wn, K) in enumerate([("w1", F_IN), ("w2", HC), ("w3", HC)]):
                tiles = []
                for kh in range(K // 128):
                    t16 = cp.tile([128, 4 * HC], F16, name=f"w16_{l}_{kh}",
                                  tag="w16stage")
                    nc.sync.dma_start(
                        t16[:], sec2d(wn, 128, 4 * HC,
                                      sub_off=kh * 128 * 4 * HC))
                    t = cp.tile([128, 4 * HC], F32, name=f"s_W{l}_{kh}")
                    nc.vector.tensor_copy(t[:], t16[:])
                    tiles.append(t)
                s_W.append(tiles)
            s_B = []
            o_b = wsec_off("b")
            for l in range(3):
                t16 = cp.tile([128, 4 * HC], F16, name=f"b16_{l}",
                              tag="w16stage")
                nc.gpsimd.dma_start(t16[:], bass.AP(
                    tensor=wbuf[:].tensor, offset=o_b + l * 4 * HC,
                    ap=[[0, 128], [1, 4 * HC]]))
                t = cp.tile([128, 4 * HC], F32, name=f"s_B{l}")
                nc.vector.tensor_copy(t[:], t16[:])
                s_B.append(t)
            s_EW = cp.tile([ED, 3 * HC], F16, name="s_EW")
            nc.sync.dma_start(s_EW[:], sec2d("ew", ED, 3 * HC))
            bnp16 = cp.tile([128, 12], F16, name="bnp16")
            nc.sync.dma_start(bnp16[:], sec2d("bnp", 128, 12))
            s_BNP = cp.tile([128, 12], F32, name="s_BNP")
            nc.vector.tensor_copy(s_BNP[:], bnp16[:])
            rc16 = cp.tile([128, G], F16, name="rc16")
            nc.gpsimd.dma_start(rc16[:], bass.AP(
                tensor=wbuf[:].tensor, offset=wsec_off("rc"),
                ap=[[0, 128], [1, G]]))
            s_RC = cp.tile([128, G], F32, name="s_RC")
            nc.vector.tensor_copy(s_RC[:], rc16[:])

            NBP = NB + NB % 2
            dl8 = cp.tile([128, NBP], mybir.dt.int8, name="dl8")
            nc.sync.dma_start(dl8[:], bass.AP(
                tensor=PKt, offset=SEC["dl"][0],
                ap=[[NBP // 2, 128], [1, NBP // 2]]).bitcast(mybir.dt.int8))
            s_dstl = cp.tile([128, NB], F32, name="s_dstl")
            nc.vector.tensor_copy(s_dstl[:], dl8[:, 0:NB])
            # int16 index sections: [16, C] on wire -> replicate to 128 rows
            C = EP // 16
            s_sidx = cp.tile([128, C], I16, name="s_sidx")
            nc.gpsimd.dma_start(s_sidx[:], bass.AP(
                tensor=PKt, offset=SEC["si"][0],
                ap=[[0, 8], [C, 16], [1, C]]).bitcast(I16))
            CQ = NPL // 16
            s_qidx = cp.tile([128, CQ], I16, name="s_qidx")
            nc.gpsimd.dma_start(s_qidx[:], bass.AP(
                tensor=PKt, offset=SEC["qi"][0],
                ap=[[0, 8], [CQ, 16], [1, CQ]]).bitcast(I16))

            # ------------------------------------------- x AllGather (layer 1)
            nc.sync.dma_start(
                cc1[:].rearrange("a b -> (a b)").unsqueeze(0),
                bass.AP(tensor=PKt, offset=SEC["x"][0],
                        ap=[[1, 1], [1, F_IN * NPL]]))
            nc.gpsimd.collective_compute(
                "AllGather", AG.bypass, replica_groups=groups,
                ins=[cc1[:]], outs=[xg1[:]])

            sbn = [None, None, None]
            tbn = [None, None, None]
            sbn3 = tbn3 = None

            # ======================================================= layers
            for l in range(3):
                if _stopped(f"l{l}proj"):
                    break
                H = 4 if l == 0 else 1
                C_h = 64 if l == 0 else HC
                inv_sqrt_c = 1.0 / math.sqrt(C_h)
                K = F_IN if l == 0 else HC
                src_g = xg1 if l == 0 else xg[l - 1]

                # ---- projections: q/k/v/s tables for all NT nodes
                with (tc.tile_pool(name=f"pj{l}", bufs=3) as wp,
                      tc.tile_pool(name=f"pjp{l}", bufs=2, space="PSUM") as pp):
                    for r in range(P):
                        for c in range(NPL // 512):
                            xts = []
                            for kh in range(K // 128):
                                if l == 0:
                                    x16 = wp.tile([128, 512], F16,
                                                  tag=f"x16_{kh}")
                                    nc.sync.dma_start(
                                        x16[:],
                                        src_g[r, kh * 128:(kh + 1) * 128,
                                              c * 512:(c + 1) * 512])
                                    xb = wp.tile([128, 512], F32,
                                                 tag=f"xb{kh}")
                                    nc.vector.tensor_copy(xb[:], x16[:])
                                    xts.append(xb)
                                else:
                                    xt = wp.tile([128, 512], F32,
                                                 tag=f"xt{kh}")
                                    nc.sync.dma_start(
                                        xt[:],
                                        src_g[r, kh * 128:(kh + 1) * 128,
                                              c * 512:(c + 1) * 512])
                                    xb = wp.tile([128, 512], F32,
                                                 tag=f"xb{kh}")
                                    nc.scalar.activation(
                                        xb[:], xt[:], AF.Identity,
                                        bias=tbn[l][:, kh:kh + 1],
                                        scale=sbn[l][:, kh:kh + 1])
                                    xts.append(xb)
                            for sub in range(4):
                                row0 = r * NPL + c * 512 + sub * 128
                                for half in range(2):
                                    ppt = pp.tile(
                                        [128, 512], F32, tag=f"pp{half}",
                                        name=f"pp{l}_{r}_{c}_{sub}_{half}")
                                    nkh = K // 128
                                    for kh in range(nkh):
                                        nc.tensor.matmul(
                                            ppt[:],
                                            xts[kh][:, sub * 128:(sub + 1) * 128],
                                            s_W[l][kh][:, half * 512:(half + 1) * 512],
                                            start=(kh == 0),
                                            stop=(kh == nkh - 1))
                                    ob = wp.tile([128, 512], F32, tag="ob")
                                    nc.vector.tensor_tensor(
                                        ob[:], ppt[:],
                                        s_B[l][:, half * 512:(half + 1) * 512],
                                        AG.add)
                                    if half == 0:
                                        nc.sync.dma_start(
                                            qtab[l][row0:row0 + 128, :],
                                            ob[:, 0:HC])
                                        nc.sync.dma_start(
                                            ktab[l][row0:row0 + 128, :],
                                            ob[:, HC:2 * HC])
                                    else:
                                        nc.sync.dma_start(
                                            vtab[l][row0:row0 + 128, :],
                                            ob[:, 0:HC])
                                        nc.sync.dma_start(
                                            stab[l][row0:row0 + 128, :],
                                            ob[:, HC:2 * HC])

                # ---- edge phase: per-window segment softmax + aggregation
                if _stopped(f"l{l}edge"):
                    break
                ewsl = s_EW[:, l * HC:(l + 1) * HC]
                with (tc.tile_pool(name=f"ed{l}", bufs=3) as wp,
                      tc.tile_pool(name=f"edp{l}", bufs=2, space="PSUM") as pe):
                    for w in range(NWIN):
                        qwg = wp.tile([128, 1, HC], F32, tag="qw")
                        nc.gpsimd.dma_gather(
                            qwg[:], qtab[l][:], s_qidx[:, w * 8:(w + 1) * 8],
                            128, 128, HC)
                        swg = wp.tile([128, 1, HC], F32, tag="sw")
                        nc.gpsimd.dma_gather(
                            swg[:], stab[l][:], s_qidx[:, w * 8:(w + 1) * 8],
                            128, 128, HC)
                        qw = qwg[:].rearrange("p a f -> p (a f)")
                        sw = swg[:].rearrange("p a f -> p (a f)")
                        nb = nblk[w]
                        pagg = (pe.tile([128, HC + H], F32, tag="agg",
                                        name=f"pagg{l}_{w}")
                                if nb > 0 else None)
                        for b in range(nb):
                            blk = wstart[w] + b
                            S = wp.tile([128, 128], F32, tag="S")
                            nc.vector.tensor_scalar(
                                S[:], io32[:], s_dstl[:, blk:blk + 1], None,
                                AG.is_equal)
                            pS = pe.tile([128, 128], F32, tag="pS")
                            nc.tensor.transpose(pS[:], S[:], s_eye[:])
                            ST = wp.tile([128, 128], F32, tag="ST")
                            nc.scalar.copy(ST[:], pS[:])
                            kg = wp.tile([128, 1, HC], F32, tag="kg")
                            nc.gpsimd.dma_gather(
                                kg[:], ktab[l][:],
                                s_sidx[:, blk * 8:(blk + 1) * 8], 128, 128, HC)
                            vg = wp.tile([128, 1, HC], F32, tag="vg")
                            nc.gpsimd.dma_gather(
                                vg[:], vtab[l][:],
                                s_sidx[:, blk * 8:(blk + 1) * 8], 128, 128, HC)
                            kg2 = kg[:].rearrange("p a f -> p (a f)")
                            vg2 = vg[:].rearrange("p a f -> p (a f)")
                            eat = wp.tile([ED, 128], F16, tag="eat")
                            nc.sync.dma_start(
                                eat[:], sec2d("ea", ED, EP)[
                                    :, blk * 128:(blk + 1) * 128])
                            pE = pe.tile([128, HC], F32, tag="pE")
                            nc.tensor.matmul(pE[:], eat[:], ewsl,
                                             start=True, stop=True)
                            pQ = pe.tile([128, HC], F32, tag="pQ")
                            nc.tensor.matmul(pQ[:], ST[:], qw,
                                             start=True, stop=True)
                            kj = wp.tile([128, HC], F32, tag="kj")
                            nc.vector.tensor_tensor(kj[:], kg2, pE[:], AG.add)
                            prod = wp.tile([128, HC], F32, tag="prod")
                            nc.vector.tensor_tensor(prod[:], pQ[:], kj[:],
                                                    AG.mult)
                            al = wp.tile([128, H], F32, tag="al")
                            nc.vector.tensor_reduce(
                                al[:],
                                prod[:].rearrange("p (h c) -> p h c", h=H),
                                mybir.AxisListType.X, AG.add)
                            ex = wp.tile([128, H], F32, tag="ex")
                            nc.scalar.activation(ex[:], al[:], AF.Exp,
                                                 scale=inv_sqrt_c)
                            vj = wp.tile([128, HC], F32, tag="vj")
                            nc.vector.tensor_tensor(vj[:], vg2, pE[:], AG.add)
                            mv = wp.tile([128, HC + H], F32, tag="mv")
                            if H == 1:
                                nc.vector.tensor_scalar_mul(mv[:, 0:HC], vj[:],
                                                            ex[:, 0:1])
                            else:
                                nc.vector.tensor_tensor(
                                    mv[:, 0:HC].rearrange(
                                        "p (h c) -> p h c", h=H),
                                    vj[:].rearrange("p (h c) -> p h c", h=H),
                                    ex[:].unsqueeze(2).broadcast_to(
                                        [128, H, C_h]),
                                    AG.mult)
                            nc.vector.tensor_copy(mv[:, HC:HC + H], ex[:])
                            nc.tensor.matmul(pagg[:, 0:HC + H], S[:], mv[:],
                                             start=(b == 0),
                                             stop=(b == nb - 1))
                        # window evacuation
                        ob2 = wp.tile([128, HC], F32, tag="ob2")
                        if nb == 0:
                            nc.vector.tensor_copy(ob2[:], sw)
                        else:
                            den = wp.tile([128, H], F32, tag="den")
                            nc.vector.tensor_scalar_add(
                                den[:], pagg[:, HC:HC + H], 1e-16)
                            rc = wp.tile([128, H], F32, tag="rc")
                            nc.vector.reciprocal(rc[:], den[:])
                            ob = wp.tile([128, HC], F32, tag="ob")
                            if H == 1:
                                nc.vector.tensor_scalar_mul(
                                    ob[:], pagg[:, 0:HC], rc[:, 0:1])
                            else:
                                nc.vector.tensor_tensor(
                                    ob[:].rearrange("p (h c) -> p h c", h=H),
                                    pagg[:, 0:HC].rearrange(
                                        "p (h c) -> p h c", h=H),
                                    rc[:].unsqueeze(2).broadcast_to(
                                        [128, H, C_h]),
                                    AG.mult)
                            nc.vector.tensor_tensor(ob2[:], ob[:], sw, AG.add)
                        if w == NWIN - 1:
                            obm = wp.tile([128, HC], F32, tag="obm")
                            nc.vector.tensor_scalar_mul(obm[:], ob2[:],
                                                        s_MASK[:, 0:1])
                        else:
                            obm = ob2
                        for fh in range(2):
                            pt = pe.tile([128, 128], F32, tag="pS",
                                         name=f"pt{l}_{w}_{fh}")
                            nc.tensor.transpose(
                                pt[:], obm[:, fh * 128:(fh + 1) * 128],
                                s_eye[:])
                            tb = wp.tile([128, 128], F32, tag="tb")
                            nc.vector.tensor_copy(tb[:], pt[:])
                            nc.sync.dma_start(
                                aggT[l][fh * 128:(fh + 1) * 128,
                                        w * 128:(w + 1) * 128], tb[:])

                # ---- AllGather pre-BN output, then global BN stats
                if _stopped(f"l{l}post"):
                    break
                nc.gpsimd.collective_compute(
                    "AllGather", AG.bypass, replica_groups=groups,
                    ins=[aggT[l][:]], outs=[xg[l][:]])

                with tc.tile_pool(name=f"st{l}", bufs=2) as wp:
                    sums = wp.tile([128, 2, P], F32, tag="sums")
                    sqs = wp.tile([128, 2, P], F32, tag="sqs")
                    for r in range(P):
                        for fh in range(2):
                            ch = wp.tile([128, NPL], F32, tag="ch")
                            nc.sync.dma_start(
                                ch[:], xg[l][r, fh * 128:(fh + 1) * 128, :])
                            nc.vector.tensor_reduce(
                                sums[:, fh:fh + 1, r:r + 1].rearrange(
                                    "p a b -> p (a b)"),
                                ch[:], mybir.AxisListType.X, AG.add)
                            scr = wp.tile([128, NPL], F32, tag="scr")
                            nc.scalar.activation(
                                scr[:], ch[:], AF.Square,
                                accum_out=sqs[:, fh:fh + 1, r:r + 1].rearrange(
                                    "p a b -> p (a b)"))
                    musum = wp.tile([128, 2], F32, tag="musum")
                    nc.vector.tensor_reduce(musum[:], sums[:],
                                            mybir.AxisListType.X, AG.add)
                    mu = cp.tile([128, 2], F32, name=f"mu{l}")
                    nc.vector.tensor_scalar_mul(mu[:], musum[:], 1.0 / N)
                    sqsum = wp.tile([128, 2], F32, tag="sqsum")
                    nc.vector.tensor_reduce(sqsum[:], sqs[:],
                                            mybir.AxisListType.X, AG.add)
                    ex2 = wp.tile([128, 2], F32, tag="ex2")
                    nc.vector.tensor_scalar_mul(ex2[:], sqsum[:], 1.0 / N)
                    m2 = wp.tile([128, 2], F32, tag="m2")
                    nc.vector.tensor_tensor(m2[:], mu[:], mu[:], AG.mult)
                    var = wp.tile([128, 2], F32, tag="var")
                    nc.vector.tensor_tensor(var[:], ex2[:], m2[:], AG.subtract)
                    vpe = wp.tile([128, 2], F32, tag="vpe")
                    nc.vector.tensor_scalar_add(vpe[:], var[:], EPS)
                    sd = wp.tile([128, 2], F32, tag="sd")
                    nc.scalar.activation(sd[:], vpe[:], AF.Sqrt)
                    rstd = wp.tile([128, 2], F32, tag="rstd")
                    nc.vector.reciprocal(rstd[:], sd[:])
                    sb_t = cp.tile([128, 2], F32, name=f"sbn{l}")
                    nc.vector.tensor_tensor(sb_t[:], rstd[:],
                                            s_BNP[:, 2 * l:2 * l + 2], AG.mult)
                    tmp = wp.tile([128, 2], F32, tag="tmp")
                    nc.vector.tensor_tensor(tmp[:], mu[:], sb_t[:], AG.mult)
                    tb_t = cp.tile([128, 2], F32, name=f"tbn{l}")
                    nc.vector.tensor_tensor(
                        tb_t[:], s_BNP[:, 6 + 2 * l:8 + 2 * l], tmp[:],
                        AG.subtract)
                    if l < 2:
                        sbn[l + 1] = sb_t
                        tbn[l + 1] = tb_t
                    else:
                        sbn3, tbn3 = sb_t, tb_t

            # =============================================== pooling (layer 3)
            with tc.tile_pool(name="pool", bufs=2) as wp:
              if not _stopped("pool"):
                padd = wp.tile([128, 2, G], F32, tag="padd")
                pmax = wp.tile([128, 2, G], F32, tag="pmax")
                for fh in range(2):
                    for r in range(P):
                        ch = wp.tile([128, NLOC], F32, tag="pch")
                        nc.sync.dma_start(
                            ch[:], xg[2][r, fh * 128:(fh + 1) * 128, 0:NLOC])
                        bnc = wp.tile([128, NLOC], F32, tag="pbn")
                        nc.scalar.activation(bnc[:], ch[:], AF.Identity,
                                             bias=tbn3[:, fh:fh + 1],
                                             scale=sbn3[:, fh:fh + 1])
                        for g, pl in enumerate(graph_parts):
                            for (pr, lo, ln) in pl:
                                if pr != r:
                                    continue
                                seg = bnc[:, lo:lo + ln]
                                if len(pl) == 1:
                                    nc.vector.tensor_reduce(
                                        padd[:, fh:fh + 1, g:g + 1].rearrange(
                                            "p a b -> p (a b)"),
                                        seg, mybir.AxisListType.X, AG.add)
                                    nc.vector.tensor_reduce(
                                        pmax[:, fh:fh + 1, g:g + 1].rearrange(
                                            "p a b -> p (a b)"),
                                        seg, mybir.AxisListType.X, AG.max)
                                else:
                                    first = (pr, lo, ln) == pl[0]
                                    sfx = "a" if first else "b"
                                    ta = wp.tile([128, 1], F32,
                                                 tag=f"t{sfx}_add",
                                                 name=f"t{sfx}a_{fh}_{g}")
                                    nc.vector.tensor_reduce(
                                        ta[:], seg, mybir.AxisListType.X,
                                        AG.add)
                                    tm = wp.tile([128, 1], F32,
                                                 tag=f"t{sfx}_max",
                                                 name=f"t{sfx}m_{fh}_{g}")
                                    nc.vector.tensor_reduce(
                                        tm[:], seg, mybir.AxisListType.X,
                                        AG.max)
                                    if first:
                                        holds[(fh, g)] = (ta, tm)
                                    else:
                                        ha, hm = holds.pop((fh, g))
                                        nc.vector.tensor_tensor(
                                            padd[:, fh:fh + 1,
                                                 g:g + 1].rearrange(
                                                "p a b -> p (a b)"),
                                            ha[:], ta[:], AG.add)
                                        nc.vector.tensor_tensor(
                                            pmax[:, fh:fh + 1,
                                                 g:g + 1].rearrange(
                                                "p a b -> p (a b)"),
                                            hm[:], tm[:], AG.max)
                pmean = wp.tile([128, 2, G], F32, tag="pmean")
                for fh in range(2):
                    nc.vector.tensor_tensor(
                        pmean[:, fh, :], padd[:, fh, :], s_RC[:], AG.mult)
                h16 = wp.tile([128, 3 * 2 * G], F16, tag="h16")
                nc.vector.tensor_copy(
                    h16[:, 0:2 * G], padd[:].rearrange("p a g -> p (a g)"))
                nc.vector.tensor_copy(
                    h16[:, 2 * G:4 * G], pmax[:].rearrange("p a g -> p (a g)"))
                nc.vector.tensor_copy(
                    h16[:, 4 * G:6 * G], pmean[:].rearrange("p a g -> p (a g)"))
                nc.sync.dma_start(hout[:], h16[:])
    nc.finalize()
    return nc


# ---------------------------------------------------------------- jit runner
class _Runner:
    """Build the PJRT executable once; each call = H2D + execute + D2H."""

    def __init__(self, nc, n_cores):
        from concourse.bass2jax import (install_neuronx_cc_hook, _bass_exec_p,
                                        partition_id_tensor)
        install_neuronx_cc_hook()
        self.nc = nc
        partition_name = (nc.partition_id_tensor.name
                          if nc.partition_id_tensor else None)
        in_names, out_names, out_avals, zero_shapes = [], [], [], []
        for alloc in nc.m.functions[0].allocations:
            if not isinstance(alloc, mybir.MemoryLocationSet):
                continue
            name = alloc.memorylocations[0].name
            if alloc.kind == "ExternalInput":
                if name != partition_name:
                    in_names.append(name)
            elif alloc.kind == "ExternalOutput":
                out_names.append(name)
                shape = tuple(alloc.tensor_shape)
                dtype = mybir.dt.np(alloc.dtype)
                out_avals.append(jax.core.ShapedArray(shape, dtype))
                zero_shapes.append((shape, dtype))
        self.in_names, self.out_names = in_names, out_names
        self.zero_shapes = zero_shapes
        n_params, n_outs = len(in_names), len(out_avals)
        all_names = (list(in_names) + list(out_names)
                     + ([partition_name] if partition_name else []))

        def _body(*args):
            operands = list(args)
            if partition_name is not None:
                operands.append(partition_id_tensor())
            outs = _bass_exec_p.bind(
                *operands, out_avals=tuple(out_avals),
                in_names=tuple(all_names), out_names=tuple(out_names),
                lowering_input_output_aliases=(), sim_require_finite=True,
                sim_require_nnan=True, nc=nc)
            return tuple(outs)

        devices = jax.devices()[:n_cores]
        mesh = Mesh(np.asarray(devices), ("core",))
        in_specs = (PartitionSpec("core"),) * (n_params + n_outs)
        out_specs = (PartitionSpec("core"),) * n_outs
        self.n_cores = n_cores
        self.fn = jax.jit(
            shard_map(_body, mesh=mesh, in_specs=in_specs,
                      out_specs=out_specs, check_rep=False),
            donate_argnums=tuple(range(n_params, n_params + n_outs)),
            keep_unused=True)

    def __call__(self, in_maps):
        if getattr(self, "_cin_src", None) is in_maps:
            concat_in = self._cin
        else:
            per_core = [[np.asarray(m[n]) for n in self.in_names]
                        for m in in_maps]
            concat_in = [np.concatenate(
                [per_core[c][i] for c in range(self.n_cores)], axis=0)
                for i in range(len(self.in_names))]
            self._cin_src, self._cin = in_maps, concat_in
        zeros = [np.zeros((self.n_cores * s[0], *s[1:]), d)
                 for s, d in self.zero_shapes]
        outs = self.fn(*concat_in, *zeros)
        outs = [np.asarray(o) for o in outs]
        return [{n: outs[i].reshape(self.n_cores, *self.zero_shapes[i][0])[c]
                 for i, n in enumerate(self.out_names)}
                for c in range(self.n_cores)]


# --------------------------------------------------------------------- kernel
def kernel(x, edge_index, edge_attr, batch,
           q1w, q1b, k1w, k1b, v1w, v1b, e1w, s1w, s1b, bn1w, bn1b,
           q2w, q2b, k2w, k2b, v2w, v2b, e2w, s2w, s2b, bn2w, bn2b,
           q3w, q3b, k3w, k3b, v3w, v3b, e3w, s3w, s3b, bn3w, bn3b,
           m1w, m1b, pa, m2w, m2b):
    global LAST_EXEC_NS
    x = np.asarray(x, np.float32)
    edge_index = np.asarray(edge_index)
    edge_attr = np.asarray(edge_attr, np.float32)
    batch = np.asarray(batch)
    src, dst = edge_index[0], edge_index[1]

    nblk, wstart, ea_l, dl_l, si_l = _pack_edges(src, dst, edge_attr)
    NB = wstart[-1]
    graph_parts, gcnt = _graph_segments(batch)
    SEC, TOT = _sections(NB)

    key = (tuple(nblk), tuple(tuple(p) for pl in graph_parts for p in pl))
    if key in _CACHE:
        runner = _CACHE[key]
    else:
        nc = _build_program(nblk, wstart, NB, graph_parts)
        runner = _Runner(nc, P)
        _CACHE[key] = runner

    xp = np.zeros((NT, F_IN), np.float16)
    for m in range(P):
        xp[m * NPL:m * NPL + NLOC] = x[m * NLOC:(m + 1) * NLOC]

    def f16(a):
        return np.asarray(a, np.float16)

    com = {}   # replicated sections, flat f16
    com["w1"] = f16(np.hstack([q1w, k1w, v1w, s1w])).ravel()
    com["w2"] = f16(np.hstack([q2w, k2w, v2w, s2w])).ravel()
    com["w3"] = f16(np.hstack([q3w, k3w, v3w, s3w])).ravel()
    com["b"] = f16(np.concatenate(
        [np.hstack([q1b, k1b, v1b, s1b]), np.hstack([q2b, k2b, v2b, s2b]),
         np.hstack([q3b, k3b, v3b, s3b])]))
    com["ew"] = f16(np.hstack([e1w, e2w, e3w])).ravel()
    bnp = np.zeros((128, 12), np.float16)
    for l, (bw, bb) in enumerate([(bn1w, bn1b), (bn2w, bn2b), (bn3w, bn3b)]):
        bnp[:, 2 * l:2 * l + 2] = np.asarray(bw).reshape(2, 128).T
        bnp[:, 6 + 2 * l:8 + 2 * l] = np.asarray(bb).reshape(2, 128).T
    com["bnp"] = bnp.ravel()
    rcv = np.zeros(128, np.float16)
    rcv[0:G] = (1.0 / np.maximum(gcnt, 1)).astype(np.float16)
    com["rc"] = rcv
    com["iota"] = np.arange(128, dtype=np.float16)

    SECW, WALL = _wall_layout()
    wall = np.zeros(WALL, np.float16)
    for name, arr in com.items():
        o, ne = SECW[name]
        wall[o:o + len(arr)] = arr
    WSH = WALL // 8

    in_maps = []
    for m in range(P):
        pk = np.zeros(TOT, np.float16)
        o, ne = SEC["x"]
        pk[o:o + ne] = np.ascontiguousarray(
            xp[m * NPL:(m + 1) * NPL].T).ravel()
        o, ne = SEC["ea"]
        pk[o:o + ne] = ea_l[m].ravel()
        o, ne = SEC["dl"]
        pk[o:o + ne] = dl_l[m].ravel().view(np.float16)
        o, ne = SEC["si"]
        pk[o:o + ne] = si_l[m].ravel().view(np.float16)
        o, ne = SEC["qi"]
        ids = (m * NPL + np.arange(NPL)).astype(np.int16)
        pk[o:o + ne] = np.ascontiguousarray(
            ids.reshape(NPL // 16, 16).T).ravel().view(np.float16)
        o, ne = SEC["wsh"]
        pk[o:o + WSH] = wall[m * WSH:(m + 1) * WSH]
        in_maps.append({"PK": pk[None, :]})

    res = runner(in_maps)
    if os.environ.get("BASS_GNN_TIME") == "1":
        t0 = time.perf_counter_ns()
        res = runner(in_maps)
        LAST_EXEC_NS = time.perf_counter_ns() - t0

    ho = np.asarray(res[0]["hout"], np.float32)   # [128, 384]
    x_add = np.empty((G, HC), np.float32)
    x_max = np.empty((G, HC), np.float32)
    x_mean = np.empty((G, HC), np.float32)
    for i, arr in enumerate([x_add, x_max, x_mean]):
        blk = ho[:, i * 2 * G:(i + 1) * 2 * G].reshape(128, 2, G)
        arr[:, 0:128] = blk[:, 0, :].T
        arr[:, 128:256] = blk[:, 1, :].T

    h = np.concatenate([x_add, x_max, x_mean], axis=1).astype(np.float32)
    h = h @ np.asarray(m1w, np.float32) + np.asarray(m1b, np.float32)
    h = np.where(h >= 0, h, np.float32(pa) * h)
    lg = h @ np.asarray(m2w, np.float32) + np.asarray(m2b, np.float32)
    mx = lg.max(axis=1, keepdims=True)
    sh = lg - mx
    return (sh - np.log(np.exp(sh).sum(axis=1, keepdims=True))).astype(np.float32)
